# revision 1
# baseline (speedup 1.0000x reference)
"""Trainium2 Bass kernel for nn_CrossAttention_47004122087816.

Math (faithful to the reference's "buggy einsum"):
    xn   = LayerNorm(x); xnb = xn * ln_w + ln_b
    q    = (xnb @ Wq) * SCALE            [n, E]
    k, v = split(media @ Wkv)            [m, E] each
    sim  = q @ k^T                       [n, m]
    colsum[j] = sum_i softmax(sim, -1)[i, j]
    out[j, :] = colsum[j] * (v @ Wout)[j, :]

Sharding: pure data-parallel - batch b=8 over 8 NeuronCores.

Key optimizations over the original baseline:
 - Host casts x/media/weights to bf16 (halves HBM traffic) and pre-permutes
   weight rows so every HBM load has 8-16KB contiguous per-partition
   descriptors.  x/media carry rows 16p+u on partition p (coalesced loads);
   softmax rows are independent and colsum sums over all rows, so the row
   permutation needs no undo - the output store uses the same pattern.
   Output is stored bf16 and upcast on host.
 - All bulk HBM traffic runs on the gpsimd SWDGE queue (a single SWDGE
   dma_start spreads across all 16 DMA engines; HWDGE queues run DMAs with
   poor concurrency), ordered by when the pipeline needs each tensor:
   media c0 -> Wkv -> media/x alternating.  Small weights on scalar HWDGE.
 - PE emission order kv0,kv1,q0,kv2,q1,kv3,q2,q3 hides the longer q-path
   latency (load -> LN -> transpose) behind kv matmuls.
 - ln_w*SCALE folded into Wq on host; ln_b folded into a host-computed q0
   row added during the q PSUM evacuation (ScalarE, free).
 - exp runs on ScalarE with accum_out giving the softmax row-sum z for
   free; Y-matmul and colsum groups are interleaved between sim groups so
   semaphore latency never idles the PE.
 - colsum accumulates into four [1,512] PSUM rows packed at partition
   offsets 0/32/64/96 of a single PSUM bank (tile_position), leaving 7
   banks for matmul double-buffering.
 - final out[j,:] = c_j * Y[j,:] with Y = v @ Wout computed during the sim
   phase (c-independent); the tail is scatter + out-of-place scale (DVE /
   ScalarE; in-place DVE ops are pathologically slow) + SWDGE stores.
"""

import sys

for _p in ("/opt/trn_rl_repo",):
    if _p not in sys.path:
        sys.path.insert(0, _p)

import numpy as np
import ml_dtypes

import concourse.bass as bass  # noqa: F401
import concourse.tile as tile
from concourse import bacc, mybir
from concourse.bass_utils import run_bass_kernel_spmd

B = 8
N = 2048          # x rows per batch element
M = 2048          # media rows per batch element
D = 1024          # model dim
E = 512           # inner dim
P = 128           # partitions
F = 512           # one PSUM bank of fp32
CT = D // P       # 8  c-tiles (contraction over model dim)
ET = E // P       # 4  e-tiles (contraction over inner dim)
NT = N // P       # 16 row tiles
JC = M // F       # 4  column chunks of 512
RPP = N // P      # 16 rows per partition (coalesced DRAM layout)
SCALE = 64 ** -0.5
EPS = 1e-5

FP = mybir.dt.float32
BF = mybir.dt.bfloat16

AF = mybir.ActivationFunctionType
ALU = mybir.AluOpType
AX = mybir.AxisListType


def _build():
    nc = bacc.Bacc("TRN2", target_bir_lowering=False, debug=False, num_devices=B)

    x = nc.dram_tensor("x", [N, D], BF, kind="ExternalInput").ap()
    media = nc.dram_tensor("media", [M, D], BF, kind="ExternalInput").ap()
    # weights pre-permuted on host: row (p*CT + kt) holds original row (kt*P + p)
    wq = nc.dram_tensor("wq", [D, E], BF, kind="ExternalInput").ap()
    wkvk = nc.dram_tensor("wkvk", [D, E], BF, kind="ExternalInput").ap()
    wkvv = nc.dram_tensor("wkvv", [D, E], BF, kind="ExternalInput").ap()
    wout = nc.dram_tensor("wout", [E, D], BF, kind="ExternalInput").ap()
    q0 = nc.dram_tensor("q0", [P, ET], FP, kind="ExternalInput").ap()
    out = nc.dram_tensor("out", [M, D], BF, kind="ExternalOutput").ap()

    with tile.TileContext(nc) as tc:
        from contextlib import ExitStack

        with ExitStack() as ctx:
            consts = ctx.enter_context(tc.tile_pool(name="consts", bufs=1))
            acts = ctx.enter_context(tc.tile_pool(name="acts", bufs=1))
            xst = ctx.enter_context(tc.tile_pool(name="xst", bufs=4))
            mst = ctx.enter_context(tc.tile_pool(name="mst", bufs=4))
            obuf = ctx.enter_context(tc.tile_pool(name="obuf", bufs=2))
            xhp = ctx.enter_context(tc.tile_pool(name="xhp", bufs=2))
            xw = ctx.enter_context(tc.tile_pool(name="xw", bufs=2))
            mtw = ctx.enter_context(tc.tile_pool(name="mtw", bufs=2))
            expp = ctx.enter_context(tc.tile_pool(name="expp", bufs=2))
            zp = ctx.enter_context(tc.tile_pool(name="zp", bufs=3))
            small = ctx.enter_context(tc.tile_pool(name="small", bufs=6))
            psim = ctx.enter_context(tc.tile_pool(name="psim", bufs=4, space="PSUM"))
            psy = ctx.enter_context(tc.tile_pool(name="psy", bufs=3, space="PSUM"))
            pscs = ctx.enter_context(tc.tile_pool(name="pscs", bufs=1, space="PSUM"))

            wkvk_t = consts.tile([P, CT, E], BF)
            wkvv_t = consts.tile([P, CT, E], BF)
            wq_t = consts.tile([P, CT, E], BF)
            wout_t = consts.tile([P, ET, D], BF)
            q0t = consts.tile([P, ET], FP)
            eps_t = consts.tile([P, 1], FP)
            nc.vector.memset(eps_t[:], EPS)

            kT = acts.tile([P, ET, M], BF)
            vT = acts.tile([P, ET, M], BF)
            qT = acts.tile([P, ET, N], BF)
            Y = acts.tile([P, RPP * D], BF)
            scol = consts.tile([P, NT], FP)

            xv = x.rearrange("(p t) d -> p t d", t=RPP)
            mv = media.rearrange("(p t) d -> p t d", t=RPP)
            ov = out.rearrange("(p t) d -> p t d", t=RPP)

            # ------------- bulk loads: gpsimd SWDGE in need-order ------------
            # SWDGE admits only ~2 DMAs in flight, so the chain is cut into
            # 0.5MB half-chunk pieces ordered exactly by when the pipeline
            # consumes them: m0 halves -> Wkv(k) -> m1a -> Wkv(v) -> ...
            mstc: list = [[None, None] for _ in range(JC)]
            xstc: list = [[None, None] for _ in range(JC)]

            def load2(kind, c, h):
                if kind == "m":
                    t = mst.tile([P, 2 * D], BF, tag="mst", name=f"mst{c}_{h}")
                    nc.gpsimd.dma_start(
                        t[:], mv[:, 4 * c + 2 * h : 4 * c + 2 * h + 2, :]
                    )
                    mstc[c][h] = t
                else:
                    t = xst.tile([P, 2 * D], BF, tag="xst", name=f"xst{c}_{h}")
                    nc.gpsimd.dma_start(
                        t[:], xv[:, 4 * c + 2 * h : 4 * c + 2 * h + 2, :]
                    )
                    xstc[c][h] = t

            load2("m", 0, 0)
            load2("m", 0, 1)
            nc.gpsimd.dma_start(
                wkvk_t[:], wkvk.rearrange("(p kt) e -> p kt e", kt=CT)
            )
            load2("m", 1, 0)
            nc.gpsimd.dma_start(
                wkvv_t[:], wkvv.rearrange("(p kt) e -> p kt e", kt=CT)
            )
            load2("m", 1, 1)
            load2("x", 0, 0)
            load2("x", 0, 1)
            load2("m", 2, 0)
            load2("m", 2, 1)
            load2("x", 1, 0)
            load2("x", 1, 1)
            load2("m", 3, 0)
            load2("m", 3, 1)
            load2("x", 2, 0)
            load2("x", 2, 1)
            load2("x", 3, 0)
            load2("x", 3, 1)
            # small weights on the otherwise-idle scalar HWDGE queue
            nc.scalar.dma_start(wq_t[:], wq.rearrange("(p kt) e -> p kt e", kt=CT))
            nc.scalar.dma_start(q0t[:], q0)
            nc.scalar.dma_start(
                wout_t[:], wout.rearrange("(p et) d -> p et d", et=ET)
            )

            # ---------------- feed helpers -----------------------------------
            def ln_block(xin, name):
                st = small.tile([P, 2, 6], FP, tag="st", name=f"st{name}")
                for sg in range(2):
                    nc.vector.bn_stats(st[:, sg, :], xin[:, sg * 512 : (sg + 1) * 512])
                mvt = small.tile([P, 2], FP, tag="mv", name=f"mv{name}")
                nc.vector.bn_aggr(mvt[:], st[:])
                sd = small.tile([P, 1], FP, tag="sd", name=f"sd{name}")
                nc.scalar.activation(
                    sd[:], mvt[:, 1:2], func=AF.Sqrt, bias=eps_t[:], scale=1.0
                )
                rsig = small.tile([P, 1], FP, tag="rsig", name=f"rsig{name}")
                nc.vector.reciprocal(rsig[:], sd[:])
                nmr = small.tile([P, 1], FP, tag="nmr", name=f"nmr{name}")
                nc.vector.tensor_scalar(
                    nmr[:], mvt[:, 0:1], rsig[:], -1.0, ALU.mult, ALU.mult
                )
                xh = xhp.tile([P, D], BF, tag="xh", name=f"xh{name}")
                nc.scalar.activation(
                    xh[:], xin[:], func=AF.Identity, bias=nmr[:], scale=rsig[:]
                )
                return xh

            def m_transpose(c):
                mtw_c = mtw.tile([P, CT, F], BF, tag="mtw", name=f"mtw{c}")
                for u in range(4):
                    nc.sync.dma_start_transpose(
                        mtw_c[:, :, u * P : (u + 1) * P],
                        mstc[c][u // 2][:, (u % 2) * D : (u % 2 + 1) * D],
                    )
                return mtw_c

            def x_transpose(c):
                xw_c = xw.tile([P, CT, F], BF, tag="xw", name=f"xw{c}")
                for u in range(4):
                    xh = ln_block(
                        xstc[c][u // 2][:, (u % 2) * D : (u % 2 + 1) * D],
                        f"{c}_{u}",
                    )
                    nc.sync.dma_start_transpose(
                        xw_c[:, :, u * P : (u + 1) * P], xh[:]
                    )
                return xw_c

            def kv_chunk(c, mtw_c):
                for e in range(2 * ET):  # k e0..3 then v e0..3
                    w_t = wkvk_t if e < ET else wkvv_t
                    eh = e % ET
                    ps = psim.tile([P, F], FP, tag="ps", name=f"kv{c}_{e}")
                    for kt in range(CT):
                        nc.tensor.matmul(
                            ps[:],
                            lhsT=w_t[:, kt, eh * P : (eh + 1) * P],
                            rhs=mtw_c[:, kt, :],
                            start=(kt == 0),
                            stop=(kt == CT - 1),
                        )
                    if e < ET:  # k
                        nc.scalar.copy(kT[:, eh, c * F : (c + 1) * F], ps[:])
                    else:  # v
                        nc.vector.tensor_copy(
                            vT[:, eh, c * F : (c + 1) * F], ps[:]
                        )

            def q_chunk(c, xw_c):
                for dt in range(ET):
                    ps = psim.tile([P, F], FP, tag="ps", name=f"q{c}_{dt}")
                    for kt in range(CT):
                        nc.tensor.matmul(
                            ps[:],
                            lhsT=wq_t[:, kt, dt * P : (dt + 1) * P],
                            rhs=xw_c[:, kt, :],
                            start=(kt == 0),
                            stop=(kt == CT - 1),
                        )
                    nc.scalar.activation(
                        qT[:, dt, c * F : (c + 1) * F],
                        ps[:],
                        func=AF.Identity,
                        bias=q0t[:, dt : dt + 1],
                        scale=1.0,
                    )

            # ------- feed: PE order kv0,kv1,q0,kv2,q1,kv3,q2,q3 --------------
            mtw0 = m_transpose(0)
            mtw1 = m_transpose(1)
            xw0 = x_transpose(0)
            kv_chunk(0, mtw0)
            kv_chunk(1, mtw1)
            xw1 = x_transpose(1)
            q_chunk(0, xw0)
            mtw2 = m_transpose(2)
            kv_chunk(2, mtw2)
            xw2 = x_transpose(2)
            q_chunk(1, xw1)
            mtw3 = m_transpose(3)
            kv_chunk(3, mtw3)
            xw3 = x_transpose(3)
            q_chunk(2, xw2)
            q_chunk(3, xw3)

            # ---------------- sim, exp (+z via accum), colsum, Y -------------
            # colsum rows live at partition offsets 0/32/64/96 of ONE bank
            cs_all = pscs.tile([P, F], FP)
            ex_hist: list = [None, None]
            zrb_hist: list = [None, None]

            def colsum_mms(it):
                ex_t = ex_hist[it % 2]
                zrb_t = zrb_hist[it % 2]
                for jc in range(JC):
                    nc.tensor.matmul(
                        cs_all[32 * jc : 32 * jc + 1, :],
                        lhsT=zrb_t[:],
                        rhs=ex_t[:, jc * F : (jc + 1) * F],
                        start=(it == 0),
                        stop=(it == NT - 1),
                        skip_group_check=True,
                        tile_position=(0, 32 * jc),
                    )

            def sim_group(it, jc, ex, zpart):
                ps = psim.tile([P, F], FP, tag="ps", name=f"sim{it}_{jc}")
                for et in range(ET):
                    nc.tensor.matmul(
                        ps[:],
                        lhsT=qT[:, et, it * P : (it + 1) * P],
                        rhs=kT[:, et, jc * F : (jc + 1) * F],
                        start=(et == 0),
                        stop=(et == ET - 1),
                    )
                nc.scalar.activation(
                    ex[:, jc * F : (jc + 1) * F],
                    ps[:],
                    func=AF.Exp,
                    bias=0.0,
                    scale=1.0,
                    accum_out=zpart[:, jc : jc + 1],
                )

            def y_group(it, n2):
                psn = psy.tile([P, F], FP, tag="py", name=f"y{it}_{n2}")
                for et in range(ET):
                    nc.tensor.matmul(
                        psn[:],
                        lhsT=vT[:, et, it * P : (it + 1) * P],
                        rhs=wout_t[:, et, n2 * F : (n2 + 1) * F],
                        start=(et == 0),
                        stop=(et == ET - 1),
                    )
                nc.vector.tensor_copy(
                    Y[:, it * D + n2 * F : it * D + (n2 + 1) * F], psn[:]
                )

            for it in range(NT):
                ex = expp.tile([P, M], BF, tag="ex", name=f"ex{it}")
                zpart = small.tile([P, JC], FP, tag="zpt", name=f"zpt{it}")
                sim_group(it, 0, ex, zpart)
                sim_group(it, 1, ex, zpart)
                y_group(it, 0)
                sim_group(it, 2, ex, zpart)
                if it > 0:
                    colsum_mms(it - 1)
                sim_group(it, 3, ex, zpart)
                y_group(it, 1)
                z = small.tile([P, 1], FP, tag="z", name=f"z{it}")
                nc.vector.tensor_reduce(z[:], zpart[:], axis=AX.X, op=ALU.add)
                zr = small.tile([P, 1], FP, tag="zr", name=f"zr{it}")
                nc.vector.reciprocal(zr[:], z[:])
                zrb = zp.tile([P, 1], BF, tag="zrb", name=f"zrb{it}")
                nc.vector.tensor_copy(zrb[:], zr[:])
                ex_hist[it % 2] = ex
                zrb_hist[it % 2] = zrb
            colsum_mms(NT - 1)

            # ---------------- tail: scatter colsum, scale Y, store -----------
            # PSUM is not DMA-readable: one whole-bank copy to SBUF first
            # (only partitions 0/32/64/96 are meaningful), then single-column
            # scatters scol[p, jc*4+b] = cs[32*jc, b*128+p].
            csum_sb = consts.tile([P, F], FP)
            nc.scalar.copy(csum_sb[:], cs_all[:])
            for jt in range(NT):
                jc, b = jt // 4, jt % 4
                q = nc.sync if jt % 2 == 0 else nc.scalar
                q.dma_start(
                    scol[:, jt : jt + 1],
                    csum_sb[32 * jc : 32 * jc + 1, b * P : (b + 1) * P],
                )
            # out-of-place scales (in-place DVE tensor ops are ~20x slower on
            # HW) alternating DVE / ScalarE, store per tile-pair over SWDGE
            for s in range(NT // 2):
                ob = obuf.tile([P, 2 * D], BF, tag="ob", name=f"ob{s}")
                for h in range(2):
                    jt = 2 * s + h
                    ysl = Y[:, jt * D : (jt + 1) * D]
                    osl = ob[:, h * D : (h + 1) * D]
                    csl = scol[:, jt : jt + 1]
                    if jt % 2 == 0:
                        nc.vector.tensor_scalar_mul(osl, ysl, csl)
                    else:
                        nc.scalar.mul(osl, ysl, csl)
                nc.gpsimd.dma_start(ov[:, 2 * s : 2 * s + 2, :], ob[:])

    nc.compile()
    return nc


_NC_CACHE = None


def _get_nc():
    global _NC_CACHE
    if _NC_CACHE is None:
        _NC_CACHE = _build()
    return _NC_CACHE


BF_NP = ml_dtypes.bfloat16


def _run(inputs, trace=False, **kw):
    nc = _get_nc()
    ln_w = np.asarray(inputs["ln_w"], dtype=np.float32)
    ln_b = np.asarray(inputs["ln_b"], dtype=np.float32)
    Wq = np.asarray(inputs["Wq"], dtype=np.float32)
    Wkv = np.asarray(inputs["Wkv"], dtype=np.float32)
    Wout = np.asarray(inputs["Wout"], dtype=np.float32)

    def permute_rows(w):  # row (kt*P + p) -> row (p*ct + kt) for big packets
        ct = w.shape[0] // P
        return np.ascontiguousarray(
            w.reshape(ct, P, w.shape[1]).transpose(1, 0, 2).reshape(w.shape)
        )

    wq_h = permute_rows((Wq * (SCALE * ln_w)[:, None]).astype(BF_NP))
    wkvk_h = permute_rows(np.ascontiguousarray(Wkv[:, :E]).astype(BF_NP))
    wkvv_h = permute_rows(np.ascontiguousarray(Wkv[:, E:]).astype(BF_NP))
    wout_h = permute_rows(Wout.astype(BF_NP))
    q0_h = np.ascontiguousarray(
        (SCALE * (ln_b @ Wq)).astype(np.float32).reshape(ET, P).T
    )

    xs = np.asarray(inputs["x"], dtype=np.float32).astype(BF_NP)
    ms = np.asarray(inputs["media"], dtype=np.float32).astype(BF_NP)
    shared = {
        "wq": wq_h,
        "wkvk": wkvk_h,
        "wkvv": wkvv_h,
        "wout": wout_h,
        "q0": q0_h,
    }
    in_maps = [
        dict(shared, x=np.ascontiguousarray(xs[b]), media=np.ascontiguousarray(ms[b]))
        for b in range(B)
    ]
    res = run_bass_kernel_spmd(nc, in_maps, core_ids=list(range(B)), trace=trace, **kw)
    out = np.stack(
        [res.results[b]["out"].astype(np.float32) for b in range(B)], axis=0
    )
    return out, res


def kernel(**inputs) -> np.ndarray:
    out, _ = _run(inputs, trace=False)
    return out



# revision 6
# speedup vs baseline: 1.1895x; 1.1895x over previous
"""Trainium2 Bass kernel for nn_CrossAttention_47004122087816.

Math (faithful to the reference's "buggy einsum"):
    xn   = LayerNorm(x); xnb = xn * ln_w + ln_b
    q    = (xnb @ Wq) * SCALE            [n, E]
    k, v = split(media @ Wkv)            [m, E] each
    sim  = q @ k^T                       [n, m]
    colsum[j] = sum_i softmax(sim, -1)[i, j]
    out[j, :] = colsum[j] * (v @ Wout)[j, :]

Sharding: pure data-parallel - batch b=8 over 8 NeuronCores.

v2 redesign (vs the DMA-transpose baseline):
 - x and media are transposed on the HOST (layout-only prep, like the
   existing weight-row permutation), so the device loads land directly in
   the [D-part, rows-free] layout every matmul wants.  This removes all 32
   on-chip dma_start_transpose ops (~8MB of DMA traffic and the 35us
   startup serialization they caused).
 - LayerNorm is restructured to work in the transposed layout:
     q_i = r_i * (x_i @ wq' - mu_i * colsum(wq') + sigma_i * q0)
   with wq' = Wq * ln_w * SCALE, q0 = SCALE * ln_b @ Wq.  Sx and Sxx come
   from ones-vector matmuls (cheap M=1 PE work), the rank-1 corrections are
   K=1 matmuls accumulated into the q PSUM groups, and the final per-row
   scale r_i is folded into the sim-phase Exp activation's per-partition
   `scale` operand - zero extra elementwise passes over q.
 - v is never materialized: W2 = Wkv_v @ Wout is folded on the host, and
   Y = media @ W2 is computed directly (same FLOPs, one less PSUM
   evacuation pass and 2MB less SBUF).
 - sigma rows are flipped to per-partition columns with 16 tiny PE
   transposes (rhs = 1x1 identity); same trick turns the colsum PSUM rows
   into per-partition scalars in the tail, replacing 16 single-column
   scatter DMAs.
 - fp8 was evaluated (DoubleRow would halve PE time) and rejected: exp()
   amplification puts even k-only fp8 at ~2e-2 rel err, the whole gate.
 - Tail: colsum transposes -> one [128,16] copy -> 16 scaled copies
   (DVE/ScalarE alternating) -> paired 4KB-descriptor SWDGE stores.
"""

import sys

for _p in ("/opt/trn_rl_repo",):
    if _p not in sys.path:
        sys.path.insert(0, _p)

import numpy as np
import ml_dtypes

import concourse.bass as bass  # noqa: F401
import concourse.tile as tile
from concourse import bacc, mybir
from concourse.bass_utils import run_bass_kernel_spmd

B = 8
N = 2048          # x rows per batch element
M = 2048          # media rows per batch element
D = 1024          # model dim
E = 512           # inner dim
P = 128           # partitions
F = 512           # one PSUM bank of fp32
KT = D // P       # 8  contraction tiles over model dim
ET = E // P       # 4  contraction tiles over inner dim
NT = N // P       # 16 row tiles (positions)
JC = M // F       # 4  column chunks of 512
CH = N // F       # 4  position chunks of 512
SCALE = 64 ** -0.5
EPS = 1e-5

FP = mybir.dt.float32
BF = mybir.dt.bfloat16

AF = mybir.ActivationFunctionType
ALU = mybir.AluOpType
AX = mybir.AxisListType


def _build():
    nc = bacc.Bacc("TRN2", target_bir_lowering=False, debug=False, num_devices=B)

    # host layouts (see _run): xt/mt row (c*1024 + p*8 + kt) col i' holds
    # x[c*512 + i', kt*128 + p] -> per-partition 8KB contiguous loads.
    xt = nc.dram_tensor("xt", [CH * D, F], BF, kind="ExternalInput").ap()
    mt = nc.dram_tensor("mt", [CH * D, F], BF, kind="ExternalInput").ap()
    wq = nc.dram_tensor("wq", [D, E], BF, kind="ExternalInput").ap()
    wk = nc.dram_tensor("wk", [D, E], BF, kind="ExternalInput").ap()
    w2 = nc.dram_tensor("w2", [D, D], BF, kind="ExternalInput").ap()
    q0r = nc.dram_tensor("q0r", [1, E], BF, kind="ExternalInput").ap()
    wqs = nc.dram_tensor("wqs", [1, E], BF, kind="ExternalInput").ap()
    out = nc.dram_tensor("out", [M, D], BF, kind="ExternalOutput").ap()

    xtv = xt.rearrange("(c p kt) i -> c p kt i", p=P, kt=KT)
    mtv = mt.rearrange("(c p kt) i -> c p kt i", p=P, kt=KT)
    # store tile jt partition p -> HBM row p*16+jt (host unscrambles);
    # paired stores give 4KB contiguous per-partition descriptors.
    ov = out.rearrange("(p t) d -> p t d", t=NT)

    with tile.TileContext(nc) as tc:
        from contextlib import ExitStack

        with ExitStack() as ctx:
            consts = ctx.enter_context(tc.tile_pool(name="consts", bufs=1))
            acts = ctx.enter_context(tc.tile_pool(name="acts", bufs=1))
            mtp = ctx.enter_context(tc.tile_pool(name="mtp", bufs=4))
            xtp = ctx.enter_context(tc.tile_pool(name="xtp", bufs=3))
            sqp = ctx.enter_context(tc.tile_pool(name="sqp", bufs=1))
            rows = ctx.enter_context(tc.tile_pool(name="rows", bufs=2))
            expp = ctx.enter_context(tc.tile_pool(name="expp", bufs=2))
            zsp = ctx.enter_context(tc.tile_pool(name="zsp", bufs=2))
            zrbp = ctx.enter_context(tc.tile_pool(name="zrbp", bufs=2))
            obuf = ctx.enter_context(tc.tile_pool(name="obuf", bufs=2))
            pmm = ctx.enter_context(tc.tile_pool(name="pmm", bufs=3, space="PSUM"))
            pyy = ctx.enter_context(tc.tile_pool(name="pyy", bufs=1, space="PSUM"))
            pst = ctx.enter_context(tc.tile_pool(name="pst", bufs=1, space="PSUM"))
            ptp = ctx.enter_context(tc.tile_pool(name="ptp", bufs=1, space="PSUM"))
            pcs = ctx.enter_context(tc.tile_pool(name="pcs", bufs=1, space="PSUM"))

            wq_t = consts.tile([P, KT, E], BF)
            wk_t = consts.tile([P, KT, E], BF)
            w2_t = consts.tile([P, KT, D], BF)
            q0t = consts.tile([1, E], BF)
            wqt = consts.tile([1, E], BF)
            ones_t = consts.tile([P, 1], BF)
            idf = consts.tile([P, 1], FP)     # 1x1 identity slices for transposes
            eps_t = consts.tile([1, 1], FP)
            r_sb = consts.tile([P, NT], FP)   # 1/sigma per position column
            colsb = consts.tile([P, NT], FP)
            csum_sb = consts.tile([P, F], FP)

            kT = acts.tile([P, ET, M], BF)
            qT = acts.tile([P, ET, N], BF)
            Y = acts.tile([P, NT, D], BF)

            nc.vector.memset(ones_t[:], 1.0)
            nc.vector.memset(idf[:], 1.0)
            nc.vector.memset(eps_t[:], EPS)

            # ---------------- bulk loads -------------------------------------
            mts: list = []
            xts: list = []

            def load_m(c):
                t = mtp.tile([P, KT, F], BF, tag="mt", name=f"mt{c}")
                nc.gpsimd.dma_start(t[:], mtv[c])
                mts.append(t)

            def load_x(c):
                t = xtp.tile([P, KT, F], BF, tag="xt", name=f"xt{c}")
                nc.gpsimd.dma_start(t[:], xtv[c])
                xts.append(t)

            # SWDGE in need-order; wk/w2 on sync HWDGE, wq + rows on scalar.
            nc.sync.dma_start(wk_t[:], wk.rearrange("(p kt) e -> p kt e", kt=KT))
            load_m(0)
            load_x(0)
            load_m(1)
            load_x(1)
            load_m(2)
            load_x(2)
            load_m(3)
            load_x(3)
            nc.scalar.dma_start(wq_t[:], wq.rearrange("(p kt) e -> p kt e", kt=KT))
            nc.scalar.dma_start(q0t[:], q0r)
            nc.scalar.dma_start(wqt[:], wqs)
            nc.sync.dma_start(w2_t[:], w2.rearrange("(p kt) d -> p kt d", kt=KT))

            # ---------------- feed helpers -----------------------------------
            def k_chunk(c):
                for e in range(ET):
                    ps = pmm.tile([P, F], FP, tag="ps", name=f"k{c}_{e}")
                    for kt in range(KT):
                        nc.tensor.matmul(
                            ps[:],
                            lhsT=wk_t[:, kt, e * P : (e + 1) * P],
                            rhs=mts[c][:, kt, :],
                            start=(kt == 0),
                            stop=(kt == KT - 1),
                        )
                    dst = kT[:, e, c * F : (c + 1) * F]
                    if e % 2 == 0:
                        nc.scalar.copy(dst, ps[:])
                    else:
                        nc.vector.tensor_copy(dst, ps[:])

            def stats_chunk(c):
                sq = sqp.tile([P, KT, F], BF, tag="sq", name=f"sq{c}")
                nc.vector.tensor_tensor(sq[:], xts[c][:], xts[c][:], ALU.mult)
                Sx = pst.tile([1, F], FP, tag="sx", name=f"sx{c}")
                Sxx = pst.tile([1, F], FP, tag="sxx", name=f"sxx{c}")
                for kt in range(KT):
                    nc.tensor.matmul(
                        Sx[:],
                        lhsT=ones_t[:],
                        rhs=xts[c][:, kt, :],
                        start=(kt == 0),
                        stop=(kt == KT - 1),
                    )
                for kt in range(KT):
                    nc.tensor.matmul(
                        Sxx[:],
                        lhsT=ones_t[:],
                        rhs=sq[:, kt, :],
                        start=(kt == 0),
                        stop=(kt == KT - 1),
                    )
                # row math: -mu (bf16), mu^2, var, sigma (f32 + bf16)
                ngm = rows.tile([1, F], BF, tag="ngm", name=f"ngm{c}")
                nc.scalar.activation(
                    ngm[:], Sx[:], func=AF.Copy, bias=0.0, scale=-1.0 / D
                )
                m2 = rows.tile([1, F], FP, tag="m2", name=f"m2{c}")
                nc.scalar.activation(
                    m2[:], Sx[:], func=AF.Square, bias=0.0, scale=1.0 / D
                )
                vt1 = rows.tile([1, F], FP, tag="vt1", name=f"vt1{c}")
                nc.vector.tensor_scalar(vt1[:], Sxx[:], 1.0 / D, None, ALU.mult)
                varx = rows.tile([1, F], FP, tag="varx", name=f"varx{c}")
                nc.vector.tensor_tensor(varx[:], vt1[:], m2[:], ALU.subtract)
                sgf = rows.tile([1, F], FP, tag="sgf", name=f"sgf{c}")
                nc.scalar.activation(
                    sgf[:], varx[:], func=AF.Sqrt, bias=eps_t[:], scale=1.0
                )
                sgb = rows.tile([1, F], BF, tag="sgb", name=f"sgb{c}")
                nc.scalar.activation(
                    sgb[:], varx[:], func=AF.Sqrt, bias=eps_t[:], scale=1.0
                )
                return ngm, sgb, sgf

            def sig_transpose(c, sgf):
                # [1,512] sigma row -> r_sb[:, 4c:4c+4] columns via 4 tiny
                # PE transposes (rhs = 1x1 identity) + one PSUM reciprocal.
                pt = ptp.tile([P, CH], FP, tag="tp", name=f"sigT{c}")
                for u in range(CH):
                    nc.tensor.matmul(
                        pt[:, u : u + 1],
                        lhsT=sgf[0:1, u * P : (u + 1) * P],
                        rhs=idf[0:1, :],
                        is_transpose=True,
                        skip_group_check=True,
                        tile_position=(0, 0),
                    )
                nc.vector.reciprocal(r_sb[:, 4 * c : 4 * c + 4], pt[:])

            def q_chunk(c, ngm, sgb):
                for e in range(ET):
                    ps = pmm.tile([P, F], FP, tag="ps", name=f"q{c}_{e}")
                    for kt in range(KT):
                        nc.tensor.matmul(
                            ps[:],
                            lhsT=wq_t[:, kt, e * P : (e + 1) * P],
                            rhs=xts[c][:, kt, :],
                            start=(kt == 0),
                            stop=False,
                        )
                    # rank-1 corrections: - mu (x) wqsum  +  sigma (x) q0
                    nc.tensor.matmul(
                        ps[:],
                        lhsT=wqt[0:1, e * P : (e + 1) * P],
                        rhs=ngm[0:1, :],
                        start=False,
                        stop=False,
                    )
                    nc.tensor.matmul(
                        ps[:],
                        lhsT=q0t[0:1, e * P : (e + 1) * P],
                        rhs=sgb[0:1, :],
                        start=False,
                        stop=True,
                    )
                    dst = qT[:, e, c * F : (c + 1) * F]
                    if e % 2 == 0:
                        nc.vector.tensor_copy(dst, ps[:])
                    else:
                        nc.scalar.copy(dst, ps[:])

            # ---- feed: PE order k0 s0 k1 T0 q0 s1 k2 T1 q1 s2 k3 T2 q2 s3 T3 q3
            k_chunk(0)
            st0 = stats_chunk(0)
            k_chunk(1)
            sig_transpose(0, st0[2])
            q_chunk(0, st0[0], st0[1])
            st1 = stats_chunk(1)
            k_chunk(2)
            sig_transpose(1, st1[2])
            q_chunk(1, st1[0], st1[1])
            st2 = stats_chunk(2)
            k_chunk(3)
            sig_transpose(2, st2[2])
            q_chunk(2, st2[0], st2[1])
            st3 = stats_chunk(3)
            sig_transpose(3, st3[2])
            q_chunk(3, st3[0], st3[1])

            # ---------------- sim, exp (+z via accum), colsum, Y -------------
            cs_all = pcs.tile([P, F], FP)
            nc.vector.memset(cs_all[:], 0.0)
            ex_hist: list = [None, None]
            zrb_hist: list = [None, None]

            def colsum_mms(it):
                ex_t = ex_hist[it % 2]
                zrb_t = zrb_hist[it % 2]
                for jc in range(JC):
                    nc.tensor.matmul(
                        cs_all[32 * jc : 32 * jc + 1, :],
                        lhsT=zrb_t[:],
                        rhs=ex_t[:, jc * F : (jc + 1) * F],
                        start=(it == 0),
                        stop=(it == NT - 1),
                        skip_group_check=True,
                        tile_position=(0, 32 * jc),
                    )

            def sim_group(it, jc, ex, zpart):
                ps = pmm.tile([P, F], FP, tag="ps", name=f"sim{it}_{jc}")
                for et in range(ET):
                    nc.tensor.matmul(
                        ps[:],
                        lhsT=qT[:, et, it * P : (it + 1) * P],
                        rhs=kT[:, et, jc * F : (jc + 1) * F],
                        start=(et == 0),
                        stop=(et == ET - 1),
                    )
                nc.scalar.activation(
                    ex[:, jc * F : (jc + 1) * F],
                    ps[:],
                    func=AF.Exp,
                    bias=0.0,
                    scale=r_sb[:, it : it + 1],
                    accum_out=zpart[:, jc : jc + 1],
                )

            def y_group(g):
                c, mb, dh = g // 8, (g % 8) // 2, g % 2
                jt = 4 * c + mb
                psn = pyy.tile([P, F], FP, tag="py", name=f"y{g}")
                for kt in range(KT):
                    nc.tensor.matmul(
                        psn[:],
                        lhsT=mts[c][:, kt, mb * P : (mb + 1) * P],
                        rhs=w2_t[:, kt, dh * F : (dh + 1) * F],
                        start=(kt == 0),
                        stop=(kt == KT - 1),
                    )
                nc.vector.tensor_copy(Y[:, jt, dh * F : (dh + 1) * F], psn[:])

            for it in range(NT):
                ex = expp.tile([P, M], BF, tag="ex", name=f"ex{it}")
                zpart = zsp.tile([P, JC], FP, tag="zpt", name=f"zpt{it}")
                sim_group(it, 0, ex, zpart)
                sim_group(it, 1, ex, zpart)
                y_group(2 * it)
                sim_group(it, 2, ex, zpart)
                if it > 0:
                    colsum_mms(it - 1)
                sim_group(it, 3, ex, zpart)
                y_group(2 * it + 1)
                z = zsp.tile([P, 1], FP, tag="z", name=f"z{it}")
                nc.vector.tensor_reduce(z[:], zpart[:], axis=AX.X, op=ALU.add)
                zr = zsp.tile([P, 1], FP, tag="zr", name=f"zr{it}")
                nc.vector.reciprocal(zr[:], z[:])
                zrb = zrbp.tile([P, 1], BF, tag="zrb", name=f"zrb{it}")
                nc.vector.tensor_copy(zrb[:], zr[:])
                ex_hist[it % 2] = ex
                zrb_hist[it % 2] = zrb
            colsum_mms(NT - 1)

            # ---------------- tail -------------------------------------------
            nc.scalar.copy(csum_sb[:], cs_all[:])
            colT = ptp.tile([P, NT], FP, tag="tp", name="colT")
            for jt in range(NT):
                jc, bb = jt // 4, jt % 4
                nc.tensor.matmul(
                    colT[:, jt : jt + 1],
                    lhsT=csum_sb[32 * jc : 32 * jc + 1, bb * P : (bb + 1) * P],
                    rhs=idf[32 * jc : 32 * jc + 1, :],
                    is_transpose=True,
                    skip_group_check=True,
                    tile_position=(32 * jc, 0),
                )
            nc.vector.tensor_copy(colsb[:], colT[:])
            for s in range(NT // 2):
                ob = obuf.tile([P, 2, D], BF, tag="ob", name=f"ob{s}")
                for h in range(2):
                    jt = 2 * s + h
                    ysl = Y[:, jt, :]
                    osl = ob[:, h, :]
                    csl = colsb[:, jt : jt + 1]
                    if jt % 2 == 0:
                        nc.vector.tensor_scalar_mul(osl, ysl, csl)
                    else:
                        nc.scalar.mul(osl, ysl, csl)
                nc.gpsimd.dma_start(ov[:, 2 * s : 2 * s + 2, :], ob[:])

    nc.compile()
    return nc


_NC_CACHE = None


def _get_nc():
    global _NC_CACHE
    if _NC_CACHE is None:
        _NC_CACHE = _build()
    return _NC_CACHE


BF_NP = ml_dtypes.bfloat16


def _prep(inputs):
    ln_w = np.asarray(inputs["ln_w"], dtype=np.float32)
    ln_b = np.asarray(inputs["ln_b"], dtype=np.float32)
    Wq = np.asarray(inputs["Wq"], dtype=np.float32)
    Wkv = np.asarray(inputs["Wkv"], dtype=np.float32)
    Wout = np.asarray(inputs["Wout"], dtype=np.float32)

    def permute_rows(w):  # row (kt*P + p) -> row (p*KT + kt) for big packets
        ct = w.shape[0] // P
        return np.ascontiguousarray(
            w.reshape(ct, P, w.shape[1]).transpose(1, 0, 2).reshape(w.shape)
        )

    wq_f = Wq * (SCALE * ln_w)[:, None]
    wq_h = permute_rows(wq_f.astype(BF_NP))
    wk_h = permute_rows(np.ascontiguousarray(Wkv[:, :E]).astype(BF_NP))
    w2_h = permute_rows((Wkv[:, E:] @ Wout).astype(BF_NP))
    q0_h = np.ascontiguousarray(
        (SCALE * (ln_b @ Wq)).astype(BF_NP).reshape(1, E)
    )
    wqs_h = np.ascontiguousarray(wq_f.sum(0).astype(BF_NP).reshape(1, E))

    def t_chunks(a):  # [2048, 1024] -> [(c p kt), i'] = [4096, 512]
        at = np.ascontiguousarray(a.astype(BF_NP).T)          # [D, n]
        return np.ascontiguousarray(
            at.reshape(KT, P, CH, F).transpose(2, 1, 0, 3).reshape(CH * D, F)
        )

    xs = np.asarray(inputs["x"], dtype=np.float32)
    ms = np.asarray(inputs["media"], dtype=np.float32)
    shared = {"wq": wq_h, "wk": wk_h, "w2": w2_h, "q0r": q0_h, "wqs": wqs_h}
    return [
        dict(shared, xt=t_chunks(xs[b]), mt=t_chunks(ms[b])) for b in range(B)
    ]


def _unscramble(o):  # [2048, 1024] HBM rows p*16+jt -> position rows jt*128+p
    return np.ascontiguousarray(
        o.reshape(P, NT, D).transpose(1, 0, 2).reshape(M, D)
    ).astype(np.float32)


def _run(inputs, trace=False, **kw):
    nc = _get_nc()
    in_maps = _prep(inputs)
    res = run_bass_kernel_spmd(nc, in_maps, core_ids=list(range(B)), trace=trace, **kw)
    out = np.stack(
        [_unscramble(res.results[b]["out"]) for b in range(B)], axis=0
    )
    return out, res


def kernel(**inputs) -> np.ndarray:
    out, _ = _run(inputs, trace=False)
    return out


# revision 10
# speedup vs baseline: 1.2243x; 1.0292x over previous
"""Trainium2 Bass kernel for nn_CrossAttention_47004122087816.

Math (faithful to the reference's "buggy einsum"):
    xn   = LayerNorm(x); xnb = xn * ln_w + ln_b
    q    = (xnb @ Wq) * SCALE            [n, E]
    k, v = split(media @ Wkv)            [m, E] each
    sim  = q @ k^T                       [n, m]
    colsum[j] = sum_i softmax(sim, -1)[i, j]
    out[j, :] = colsum[j] * (v @ Wout)[j, :]

Sharding: pure data-parallel - batch b=8 over 8 NeuronCores.

v2 redesign (vs the DMA-transpose baseline):
 - x and media are transposed on the HOST (layout-only prep, like the
   existing weight-row permutation), so the device loads land directly in
   the [D-part, rows-free] layout every matmul wants.  This removes all 32
   on-chip dma_start_transpose ops (~8MB of DMA traffic and the 35us
   startup serialization they caused).
 - LayerNorm is restructured to work in the transposed layout:
     q_i = r_i * (x_i @ wq' - mu_i * colsum(wq') + sigma_i * q0)
   with wq' = Wq * ln_w * SCALE, q0 = SCALE * ln_b @ Wq.  Sx and Sxx come
   from ones-vector matmuls (cheap M=1 PE work), the rank-1 corrections are
   K=1 matmuls accumulated into the q PSUM groups, and the final per-row
   scale r_i is folded into the sim-phase Exp activation's per-partition
   `scale` operand - zero extra elementwise passes over q.
 - v is never materialized: W2 = Wkv_v @ Wout is folded on the host, and
   Y = media @ W2 is computed directly (same FLOPs, one less PSUM
   evacuation pass and 2MB less SBUF).
 - sigma rows are flipped to per-partition columns with 16 tiny PE
   transposes (rhs = 1x1 identity); same trick turns the colsum PSUM rows
   into per-partition scalars in the tail, replacing 16 single-column
   scatter DMAs.
 - fp8 was evaluated (DoubleRow would halve PE time) and rejected: exp()
   amplification puts even k-only fp8 at ~2e-2 rel err, the whole gate.
 - Tail: colsum transposes -> one [128,16] copy -> 16 scaled copies
   (DVE/ScalarE alternating) -> paired 4KB-descriptor SWDGE stores.
"""

import sys

for _p in ("/opt/trn_rl_repo",):
    if _p not in sys.path:
        sys.path.insert(0, _p)

import numpy as np
import ml_dtypes

import concourse.bass as bass  # noqa: F401
import concourse.tile as tile
from concourse import bacc, mybir
from concourse.bass_utils import run_bass_kernel_spmd

B = 8
N = 2048          # x rows per batch element
M = 2048          # media rows per batch element
D = 1024          # model dim
E = 512           # inner dim
P = 128           # partitions
F = 512           # one PSUM bank of fp32
KT = D // P       # 8  contraction tiles over model dim
ET = E // P       # 4  contraction tiles over inner dim
NT = N // P       # 16 row tiles (positions)
JC = M // F       # 4  column chunks of 512
CH = N // F       # 4  position chunks of 512
SCALE = 64 ** -0.5
EPS = 1e-5

FP = mybir.dt.float32
BF = mybir.dt.bfloat16

AF = mybir.ActivationFunctionType
ALU = mybir.AluOpType
AX = mybir.AxisListType


def _build():
    nc = bacc.Bacc("TRN2", target_bir_lowering=False, debug=False, num_devices=B)

    # host layouts (see _run): xt/mt row (c*1024 + p*8 + kt) col i' holds
    # x[c*512 + i', kt*128 + p] -> per-partition 8KB contiguous loads.
    xt = nc.dram_tensor("xt", [CH * D, F], BF, kind="ExternalInput").ap()
    mt = nc.dram_tensor("mt", [CH * D, F], BF, kind="ExternalInput").ap()
    wq = nc.dram_tensor("wq", [D, E], BF, kind="ExternalInput").ap()
    wk = nc.dram_tensor("wk", [D, E], BF, kind="ExternalInput").ap()
    w2 = nc.dram_tensor("w2", [D, D], BF, kind="ExternalInput").ap()
    wqq = nc.dram_tensor("wqq", [2, E], BF, kind="ExternalInput").ap()
    out = nc.dram_tensor("out", [M, D], BF, kind="ExternalOutput").ap()

    xtv = xt.rearrange("(c p kt) i -> c p kt i", p=P, kt=KT)
    mtv = mt.rearrange("(c p kt) i -> c p kt i", p=P, kt=KT)
    # store tile jt partition p -> HBM row p*16+jt (host unscrambles);
    # paired stores give 4KB contiguous per-partition descriptors.
    ov = out.rearrange("(p t) d -> p t d", t=NT)

    with tile.TileContext(nc) as tc:
        from contextlib import ExitStack

        with ExitStack() as ctx:
            consts = ctx.enter_context(tc.tile_pool(name="consts", bufs=1))
            acts = ctx.enter_context(tc.tile_pool(name="acts", bufs=1))
            mtp = ctx.enter_context(tc.tile_pool(name="mtp", bufs=4))
            xtp = ctx.enter_context(tc.tile_pool(name="xtp", bufs=3))
            sqp = ctx.enter_context(tc.tile_pool(name="sqp", bufs=1))
            rows = ctx.enter_context(tc.tile_pool(name="rows", bufs=2))
            expp = ctx.enter_context(tc.tile_pool(name="expp", bufs=2))
            zsp = ctx.enter_context(tc.tile_pool(name="zsp", bufs=2))
            zrbp = ctx.enter_context(tc.tile_pool(name="zrbp", bufs=2))
            obuf = ctx.enter_context(tc.tile_pool(name="obuf", bufs=3))
            pmm = ctx.enter_context(tc.tile_pool(name="pmm", bufs=3, space="PSUM"))
            pyy = ctx.enter_context(tc.tile_pool(name="pyy", bufs=1, space="PSUM"))
            pst = ctx.enter_context(tc.tile_pool(name="pst", bufs=1, space="PSUM"))
            ptp = ctx.enter_context(tc.tile_pool(name="ptp", bufs=1, space="PSUM"))
            pcs = ctx.enter_context(tc.tile_pool(name="pcs", bufs=1, space="PSUM"))

            wq_t = consts.tile([P, KT, E], BF)
            wk_t = consts.tile([P, KT, E], BF)
            w2_t = consts.tile([P, KT, D], BF)
            wqq_t = consts.tile([2, E], BF)
            wdum = consts.tile([P, F], BF)
            ones_t = consts.tile([P, 1], BF)
            idf = consts.tile([P, 1], FP)     # 1x1 identity slices for transposes
            eps_t = consts.tile([1, 1], FP)
            r_sb = consts.tile([P, NT], FP)   # 1/sigma per position column
            colsb = consts.tile([P, NT], FP)
            csum_sb = consts.tile([P, F], FP)

            kT = acts.tile([P, ET, M], BF)
            qT = acts.tile([P, ET, N], BF)
            Y = acts.tile([P, NT, D], BF)

            nc.vector.memset(ones_t[:], 1.0)
            nc.vector.memset(idf[:], 1.0)
            nc.vector.memset(eps_t[:], EPS)
            nc.vector.memset(wdum[:], 0.125)
            # PE p-state warm-up during the initial DMA window
            for wdi in range(5):
                pd = pmm.tile([P, F], FP, tag="ps", name=f"warm{wdi}")
                nc.tensor.matmul(
                    pd[:], lhsT=wdum[:, 0:P], rhs=wdum[:], start=True, stop=True
                )

            # ---------------- bulk loads -------------------------------------
            mts: list = []
            xts: list = []

            def load_m(c):
                t = mtp.tile([P, KT, F], BF, tag="mt", name=f"mt{c}")
                nc.gpsimd.dma_start(t[:], mtv[c])
                mts.append(t)

            def load_x(c):
                t = xtp.tile([P, KT, F], BF, tag="xt", name=f"xt{c}")
                nc.gpsimd.dma_start(t[:], xtv[c])
                xts.append(t)

            # SWDGE in need-order; wk/wq/wqq on scalar HWDGE; w2 on sync,
            # emitted after k_chunk(0) so k0's queue-semaphore wait
            # does not cover it.
            nc.scalar.dma_start(wk_t[:], wk.rearrange("(p kt) e -> p kt e", kt=KT))
            load_m(0)
            load_x(0)
            load_m(1)
            load_x(1)
            load_m(2)
            load_x(2)
            load_m(3)
            load_x(3)
            nc.scalar.dma_start(wq_t[:], wq.rearrange("(p kt) e -> p kt e", kt=KT))
            nc.scalar.dma_start(wqq_t[:], wqq)

            # ---------------- feed helpers -----------------------------------
            def k_chunk(c):
                for e in range(ET):
                    ps = pmm.tile([P, F], FP, tag="ps", name=f"k{c}_{e}")
                    for kt in range(KT):
                        nc.tensor.matmul(
                            ps[:],
                            lhsT=wk_t[:, kt, e * P : (e + 1) * P],
                            rhs=mts[c][:, kt, :],
                            start=(kt == 0),
                            stop=(kt == KT - 1),
                        )
                    dst = kT[:, e, c * F : (c + 1) * F]
                    if e % 2 == 0:
                        nc.scalar.copy(dst, ps[:])
                    else:
                        nc.vector.tensor_copy(dst, ps[:])

            def stats_chunk(c):
                sq = sqp.tile([P, KT, F], BF, tag="sq", name=f"sq{c}")
                nc.vector.tensor_tensor(sq[:], xts[c][:], xts[c][:], ALU.mult)
                Sx = pst.tile([1, F], FP, tag="sx", name=f"sx{c}")
                Sxx = pst.tile([1, F], FP, tag="sxx", name=f"sxx{c}")
                for kt in range(KT):
                    nc.tensor.matmul(
                        Sx[:],
                        lhsT=ones_t[:],
                        rhs=xts[c][:, kt, :],
                        start=(kt == 0),
                        stop=(kt == KT - 1),
                    )
                for kt in range(KT):
                    nc.tensor.matmul(
                        Sxx[:],
                        lhsT=ones_t[:],
                        rhs=sq[:, kt, :],
                        start=(kt == 0),
                        stop=(kt == KT - 1),
                    )
                # row math: -mu (bf16), mu^2, var, sigma (f32 + bf16)
                m2 = rows.tile([1, F], FP, tag="m2", name=f"m2{c}")
                nc.scalar.activation(
                    m2[:], Sx[:], func=AF.Square, bias=0.0, scale=1.0 / D
                )
                vt1 = rows.tile([1, F], FP, tag="vt1", name=f"vt1{c}")
                nc.vector.tensor_scalar(vt1[:], Sxx[:], 1.0 / D, None, ALU.mult)
                varx = rows.tile([1, F], FP, tag="varx", name=f"varx{c}")
                nc.vector.tensor_tensor(varx[:], vt1[:], m2[:], ALU.subtract)
                sgf = rows.tile([1, F], FP, tag="sgf", name=f"sgf{c}")
                nc.scalar.activation(
                    sgf[:], varx[:], func=AF.Sqrt, bias=eps_t[:], scale=1.0
                )
                # stack [-mu; sigma] on partitions 0/1 for one K=2 rank-1
                st2 = rows.tile([2, F], BF, tag="st2", name=f"st2{c}")
                nc.scalar.activation(
                    st2[0:1, :], Sx[:], func=AF.Copy, bias=0.0, scale=-1.0 / D
                )
                sgb = rows.tile([1, F], BF, tag="sgb", name=f"sgb{c}")
                nc.scalar.activation(
                    sgb[:], varx[:], func=AF.Sqrt, bias=eps_t[:], scale=1.0
                )
                nc.scalar.dma_start(st2[1:2, :], sgb[:])
                return st2, sgf

            def sig_transpose(c, sgf):
                # [1,512] sigma row -> r_sb[:, 4c:4c+4] columns via 4 tiny
                # PE transposes (rhs = 1x1 identity) + one PSUM reciprocal.
                pt = ptp.tile([P, CH], FP, tag="tp", name=f"sigT{c}")
                for u in range(CH):
                    nc.tensor.matmul(
                        pt[:, u : u + 1],
                        lhsT=sgf[0:1, u * P : (u + 1) * P],
                        rhs=idf[0:1, :],
                        is_transpose=True,
                        skip_group_check=True,
                        tile_position=(0, 0),
                    )
                nc.vector.reciprocal(r_sb[:, 4 * c : 4 * c + 4], pt[:])

            def q_chunk(c, st2):
                for e in range(ET):
                    ps = pmm.tile([P, F], FP, tag="ps", name=f"q{c}_{e}")
                    for kt in range(KT):
                        nc.tensor.matmul(
                            ps[:],
                            lhsT=wq_t[:, kt, e * P : (e + 1) * P],
                            rhs=xts[c][:, kt, :],
                            start=(kt == 0),
                            stop=False,
                        )
                    # K=2 rank-1: - mu (x) wqsum  +  sigma (x) q0
                    nc.tensor.matmul(
                        ps[:],
                        lhsT=wqq_t[0:2, e * P : (e + 1) * P],
                        rhs=st2[0:2, :],
                        start=False,
                        stop=True,
                    )
                    dst = qT[:, e, c * F : (c + 1) * F]
                    if e % 2 == 0:
                        nc.vector.tensor_copy(dst, ps[:])
                    else:
                        nc.scalar.copy(dst, ps[:])

            # ---- feed: PE order k0 s0 k1 T0 q0 s1 k2 T1 q1 s2 k3 T2 q2 s3 T3 q3
            k_chunk(0)
            nc.sync.dma_start(w2_t[:], w2.rearrange("(p kt) d -> p kt d", kt=KT))
            sta = stats_chunk(0)
            k_chunk(1)
            sig_transpose(0, sta[1])
            q_chunk(0, sta[0])
            stb = stats_chunk(1)
            k_chunk(2)
            sig_transpose(1, stb[1])
            q_chunk(1, stb[0])
            stc = stats_chunk(2)
            k_chunk(3)
            sig_transpose(2, stc[1])
            q_chunk(2, stc[0])
            std = stats_chunk(3)
            sig_transpose(3, std[1])
            q_chunk(3, std[0])

            # ---------------- sim, exp (+z via accum), colsum, Y -------------
            cs_all = pcs.tile([P, F], FP)
            nc.vector.memset(cs_all[:], 0.0)
            ex_hist: list = [None, None]
            zrb_hist: list = [None, None]

            def colsum_mms(it):
                ex_t = ex_hist[it % 2]
                zrb_t = zrb_hist[it % 2]
                for jc in range(JC):
                    nc.tensor.matmul(
                        cs_all[32 * jc : 32 * jc + 1, :],
                        lhsT=zrb_t[:],
                        rhs=ex_t[:, jc * F : (jc + 1) * F],
                        start=(it == 0),
                        stop=(it == NT - 1),
                        skip_group_check=True,
                        tile_position=(0, 32 * jc),
                    )

            def sim_group(it, jc, ex, zpart):
                ps = pmm.tile([P, F], FP, tag="ps", name=f"sim{it}_{jc}")
                for et in range(ET):
                    nc.tensor.matmul(
                        ps[:],
                        lhsT=qT[:, et, it * P : (it + 1) * P],
                        rhs=kT[:, et, jc * F : (jc + 1) * F],
                        start=(et == 0),
                        stop=(et == ET - 1),
                    )
                nc.scalar.activation(
                    ex[:, jc * F : (jc + 1) * F],
                    ps[:],
                    func=AF.Exp,
                    bias=0.0,
                    scale=r_sb[:, it : it + 1],
                    accum_out=zpart[:, jc : jc + 1],
                )

            def y_group(g):
                c, mb, dh = g // 8, (g % 8) // 2, g % 2
                jt = 4 * c + mb
                psn = pyy.tile([P, F], FP, tag="py", name=f"y{g}")
                for kt in range(KT):
                    nc.tensor.matmul(
                        psn[:],
                        lhsT=mts[c][:, kt, mb * P : (mb + 1) * P],
                        rhs=w2_t[:, kt, dh * F : (dh + 1) * F],
                        start=(kt == 0),
                        stop=(kt == KT - 1),
                    )
                nc.vector.tensor_copy(Y[:, jt, dh * F : (dh + 1) * F], psn[:])

            for it in range(NT):
                ex = expp.tile([P, M], BF, tag="ex", name=f"ex{it}")
                zpart = zsp.tile([P, JC], FP, tag="zpt", name=f"zpt{it}")
                sim_group(it, 0, ex, zpart)
                sim_group(it, 1, ex, zpart)
                y_group(2 * it)
                sim_group(it, 2, ex, zpart)
                if it > 0:
                    colsum_mms(it - 1)
                sim_group(it, 3, ex, zpart)
                y_group(2 * it + 1)
                z = zsp.tile([P, 1], FP, tag="z", name=f"z{it}")
                nc.vector.tensor_reduce(z[:], zpart[:], axis=AX.X, op=ALU.add)
                zr = zsp.tile([P, 1], FP, tag="zr", name=f"zr{it}")
                nc.vector.reciprocal(zr[:], z[:])
                zrb = zrbp.tile([P, 1], BF, tag="zrb", name=f"zrb{it}")
                nc.vector.tensor_copy(zrb[:], zr[:])
                ex_hist[it % 2] = ex
                zrb_hist[it % 2] = zrb
            colsum_mms(NT - 1)

            # ---------------- tail -------------------------------------------
            # per jc-chunk: 4 transposes -> colsb columns -> 4 scales -> 2
            # paired stores, triggers alternating gpsimd/sync queues; scales
            # DVE-heavy (ScalarE copies are 2.5x slower).
            nc.scalar.copy(csum_sb[:], cs_all[:])
            colT = ptp.tile([P, NT], FP, tag="tp", name="colT")
            for jc in range(JC):
                for bb in range(4):
                    jt = 4 * jc + bb
                    nc.tensor.matmul(
                        colT[:, jt : jt + 1],
                        lhsT=csum_sb[32 * jc : 32 * jc + 1, bb * P : (bb + 1) * P],
                        rhs=idf[32 * jc : 32 * jc + 1, :],
                        is_transpose=True,
                        skip_group_check=True,
                        tile_position=(32 * jc, 0),
                    )
                nc.vector.tensor_copy(
                    colsb[:, 4 * jc : 4 * jc + 4], colT[:, 4 * jc : 4 * jc + 4]
                )
                for sh in range(2):
                    s = 2 * jc + sh
                    ob = obuf.tile([P, 2, D], BF, tag="ob", name=f"ob{s}")
                    for h in range(2):
                        jt = 2 * s + h
                        ysl = Y[:, jt, :]
                        osl = ob[:, h, :]
                        csl = colsb[:, jt : jt + 1]
                        if jt % 4 == 1:
                            nc.scalar.mul(osl, ysl, csl)
                        else:
                            nc.vector.tensor_scalar_mul(osl, ysl, csl)
                    q = nc.gpsimd if s % 2 == 0 else nc.sync
                    q.dma_start(ov[:, 2 * s : 2 * s + 2, :], ob[:])

    nc.compile()
    return nc


_NC_CACHE = None


def _get_nc():
    global _NC_CACHE
    if _NC_CACHE is None:
        _NC_CACHE = _build()
    return _NC_CACHE


BF_NP = ml_dtypes.bfloat16


def _prep(inputs):
    ln_w = np.asarray(inputs["ln_w"], dtype=np.float32)
    ln_b = np.asarray(inputs["ln_b"], dtype=np.float32)
    Wq = np.asarray(inputs["Wq"], dtype=np.float32)
    Wkv = np.asarray(inputs["Wkv"], dtype=np.float32)
    Wout = np.asarray(inputs["Wout"], dtype=np.float32)

    def permute_rows(w):  # row (kt*P + p) -> row (p*KT + kt) for big packets
        ct = w.shape[0] // P
        return np.ascontiguousarray(
            w.reshape(ct, P, w.shape[1]).transpose(1, 0, 2).reshape(w.shape)
        )

    wq_f = Wq * (SCALE * ln_w)[:, None]
    wq_h = permute_rows(wq_f.astype(BF_NP))
    wk_h = permute_rows(np.ascontiguousarray(Wkv[:, :E]).astype(BF_NP))
    w2_h = permute_rows((Wkv[:, E:] @ Wout).astype(BF_NP))
    wqq_h = np.ascontiguousarray(
        np.stack([wq_f.sum(0), SCALE * (ln_b @ Wq)]).astype(BF_NP)
    )

    def t_chunks(a):  # [2048, 1024] -> [(c p kt), i'] = [4096, 512]
        at = np.ascontiguousarray(a.astype(BF_NP).T)          # [D, n]
        return np.ascontiguousarray(
            at.reshape(KT, P, CH, F).transpose(2, 1, 0, 3).reshape(CH * D, F)
        )

    xs = np.asarray(inputs["x"], dtype=np.float32)
    ms = np.asarray(inputs["media"], dtype=np.float32)
    shared = {"wq": wq_h, "wk": wk_h, "w2": w2_h, "wqq": wqq_h}
    return [
        dict(shared, xt=t_chunks(xs[b]), mt=t_chunks(ms[b])) for b in range(B)
    ]


def _unscramble(o):  # [2048, 1024] HBM rows p*16+jt -> position rows jt*128+p
    return np.ascontiguousarray(
        o.reshape(P, NT, D).transpose(1, 0, 2).reshape(M, D)
    ).astype(np.float32)


def _run(inputs, trace=False, **kw):
    nc = _get_nc()
    in_maps = _prep(inputs)
    res = run_bass_kernel_spmd(nc, in_maps, core_ids=list(range(B)), trace=trace, **kw)
    out = np.stack(
        [_unscramble(res.results[b]["out"]) for b in range(B)], axis=0
    )
    return out, res


def kernel(**inputs) -> np.ndarray:
    out, _ = _run(inputs, trace=False)
    return out


# revision 14
# speedup vs baseline: 1.2319x; 1.0062x over previous
"""Trainium2 Bass kernel for nn_CrossAttention_47004122087816.

Math (faithful to the reference's "buggy einsum"):
    xn   = LayerNorm(x); xnb = xn * ln_w + ln_b
    q    = (xnb @ Wq) * SCALE            [n, E]
    k, v = split(media @ Wkv)            [m, E] each
    sim  = q @ k^T                       [n, m]
    colsum[j] = sum_i softmax(sim, -1)[i, j]
    out[j, :] = colsum[j] * (v @ Wout)[j, :]

Sharding: pure data-parallel - batch b=8 over 8 NeuronCores.

v2 redesign (vs the DMA-transpose baseline):
 - x and media are transposed on the HOST (layout-only prep, like the
   existing weight-row permutation), so the device loads land directly in
   the [D-part, rows-free] layout every matmul wants.  This removes all 32
   on-chip dma_start_transpose ops (~8MB of DMA traffic and the 35us
   startup serialization they caused).
 - LayerNorm is restructured to work in the transposed layout:
     q_i = r_i * (x_i @ wq' - mu_i * colsum(wq') + sigma_i * q0)
   with wq' = Wq * ln_w * SCALE, q0 = SCALE * ln_b @ Wq.  Sx and Sxx come
   from ones-vector matmuls (cheap M=1 PE work), the rank-1 corrections are
   K=1 matmuls accumulated into the q PSUM groups, and the final per-row
   scale r_i is folded into the sim-phase Exp activation's per-partition
   `scale` operand - zero extra elementwise passes over q.
 - v is never materialized: W2 = Wkv_v @ Wout is folded on the host, and
   Y = media @ W2 is computed directly (same FLOPs, one less PSUM
   evacuation pass and 2MB less SBUF).
 - sigma rows are flipped to per-partition columns with 16 tiny PE
   transposes (rhs = 1x1 identity); same trick turns the colsum PSUM rows
   into per-partition scalars in the tail, replacing 16 single-column
   scatter DMAs.
 - fp8 was evaluated (DoubleRow would halve PE time) and rejected: exp()
   amplification puts even k-only fp8 at ~2e-2 rel err, the whole gate.
 - Tail: colsum transposes -> one [128,16] copy -> 16 scaled copies
   (DVE/ScalarE alternating) -> paired 4KB-descriptor SWDGE stores.
"""

import sys

for _p in ("/opt/trn_rl_repo",):
    if _p not in sys.path:
        sys.path.insert(0, _p)

import numpy as np
import ml_dtypes

import concourse.bass as bass  # noqa: F401
import concourse.tile as tile
from concourse import bacc, mybir
from concourse.bass_utils import run_bass_kernel_spmd

B = 8
N = 2048          # x rows per batch element
M = 2048          # media rows per batch element
D = 1024          # model dim
E = 512           # inner dim
P = 128           # partitions
F = 512           # one PSUM bank of fp32
KT = D // P       # 8  contraction tiles over model dim
ET = E // P       # 4  contraction tiles over inner dim
NT = N // P       # 16 row tiles (positions)
JC = M // F       # 4  column chunks of 512
CH = N // F       # 4  position chunks of 512
SCALE = 64 ** -0.5
EPS = 1e-5

FP = mybir.dt.float32
BF = mybir.dt.bfloat16

AF = mybir.ActivationFunctionType
ALU = mybir.AluOpType
AX = mybir.AxisListType


def _build():
    nc = bacc.Bacc("TRN2", target_bir_lowering=False, debug=False, num_devices=B)

    # host layouts (see _run): xt/mt row (c*1024 + p*8 + kt) col i' holds
    # x[c*512 + i', kt*128 + p] -> per-partition 8KB contiguous loads.
    xt = nc.dram_tensor("xt", [CH * D, F], BF, kind="ExternalInput").ap()
    mt = nc.dram_tensor("mt", [CH * D, F], BF, kind="ExternalInput").ap()
    wq = nc.dram_tensor("wq", [D, E], BF, kind="ExternalInput").ap()
    wk = nc.dram_tensor("wk", [D, E], BF, kind="ExternalInput").ap()
    w2 = nc.dram_tensor("w2", [D, D], BF, kind="ExternalInput").ap()
    wqq = nc.dram_tensor("wqq", [2, E], BF, kind="ExternalInput").ap()
    out = nc.dram_tensor("out", [M, D], BF, kind="ExternalOutput").ap()

    xtv = xt.rearrange("(c p kt) i -> c p kt i", p=P, kt=KT)
    mtv = mt.rearrange("(c p kt) i -> c p kt i", p=P, kt=KT)
    # store tile jt partition p -> HBM row p*16+jt (host unscrambles);
    # paired stores give 4KB contiguous per-partition descriptors.
    ov = out.rearrange("(p t) d -> p t d", t=NT)

    with tile.TileContext(nc) as tc:
        from contextlib import ExitStack

        with ExitStack() as ctx:
            consts = ctx.enter_context(tc.tile_pool(name="consts", bufs=1))
            acts = ctx.enter_context(tc.tile_pool(name="acts", bufs=1))
            mtp = ctx.enter_context(tc.tile_pool(name="mtp", bufs=4))
            xtp = ctx.enter_context(tc.tile_pool(name="xtp", bufs=3))
            sqp = ctx.enter_context(tc.tile_pool(name="sqp", bufs=1))
            rows = ctx.enter_context(tc.tile_pool(name="rows", bufs=2))
            expp = ctx.enter_context(tc.tile_pool(name="expp", bufs=2))
            zsp = ctx.enter_context(tc.tile_pool(name="zsp", bufs=2))
            zrbp = ctx.enter_context(tc.tile_pool(name="zrbp", bufs=2))
            obuf = ctx.enter_context(tc.tile_pool(name="obuf", bufs=6))
            pmm = ctx.enter_context(tc.tile_pool(name="pmm", bufs=3, space="PSUM"))
            pyy = ctx.enter_context(tc.tile_pool(name="pyy", bufs=1, space="PSUM"))
            pst = ctx.enter_context(tc.tile_pool(name="pst", bufs=1, space="PSUM"))
            ptp = ctx.enter_context(tc.tile_pool(name="ptp", bufs=1, space="PSUM"))
            pcs = ctx.enter_context(tc.tile_pool(name="pcs", bufs=1, space="PSUM"))

            wq_t = consts.tile([P, KT, E], BF)
            wk_t = consts.tile([P, KT, E], BF)
            w2_t = consts.tile([P, KT, D], BF)
            wqq_t = consts.tile([2, E], BF)
            wdum = consts.tile([P, F], BF)
            ones_t = consts.tile([P, 1], BF)
            idf = consts.tile([P, 1], FP)     # 1x1 identity slices for transposes
            eps_t = consts.tile([1, 1], FP)
            r_sb = consts.tile([P, NT], FP)   # 1/sigma per position column
            colsb = consts.tile([P, NT], FP)
            csum_sb = consts.tile([P, F], FP)

            kT = acts.tile([P, ET, M], BF)
            qT = acts.tile([P, ET, N], BF)
            Y = acts.tile([P, NT, D], BF)

            nc.vector.memset(ones_t[:], 1.0)
            nc.vector.memset(idf[:], 1.0)
            nc.vector.memset(eps_t[:], EPS)
            nc.vector.memset(wdum[:], 0.125)
            # PE p-state warm-up during the initial DMA window
            for wdi in range(5):
                pd = pmm.tile([P, F], FP, tag="ps", name=f"warm{wdi}")
                nc.tensor.matmul(
                    pd[:], lhsT=wdum[:, 0:P], rhs=wdum[:], start=True, stop=True
                )

            # ---------------- bulk loads -------------------------------------
            mts: list = []
            xts: list = []

            def load_m(c):
                t = mtp.tile([P, KT, F], BF, tag="mt", name=f"mt{c}")
                nc.gpsimd.dma_start(t[:], mtv[c])
                mts.append(t)

            def load_x(c):
                t = xtp.tile([P, KT, F], BF, tag="xt", name=f"xt{c}")
                nc.gpsimd.dma_start(t[:], xtv[c])
                xts.append(t)

            # SWDGE in need-order; wk/wq/wqq on scalar HWDGE; w2 on sync,
            # emitted after k_chunk(0) so k0's queue-semaphore wait
            # does not cover it.
            nc.scalar.dma_start(wk_t[:], wk.rearrange("(p kt) e -> p kt e", kt=KT))
            load_m(0)
            load_x(0)
            load_m(1)
            load_x(1)
            load_m(2)
            load_x(2)
            load_m(3)
            load_x(3)
            # ---------------- feed helpers -----------------------------------
            def k_chunk(c):
                for e in range(ET):
                    ps = pmm.tile([P, F], FP, tag="ps", name=f"k{c}_{e}")
                    for kt in range(KT):
                        nc.tensor.matmul(
                            ps[:],
                            lhsT=wk_t[:, kt, e * P : (e + 1) * P],
                            rhs=mts[c][:, kt, :],
                            start=(kt == 0),
                            stop=(kt == KT - 1),
                        )
                    dst = kT[:, e, c * F : (c + 1) * F]
                    if e % 2 == 0:
                        nc.scalar.copy(dst, ps[:])
                    else:
                        nc.vector.tensor_copy(dst, ps[:])

            def stats_chunk(c):
                sq = sqp.tile([P, KT, F], BF, tag="sq", name=f"sq{c}")
                nc.vector.tensor_tensor(sq[:], xts[c][:], xts[c][:], ALU.mult)
                Sx = pst.tile([1, F], FP, tag="sx", name=f"sx{c}")
                Sxx = pst.tile([1, F], FP, tag="sxx", name=f"sxx{c}")
                for kt in range(KT):
                    nc.tensor.matmul(
                        Sx[:],
                        lhsT=ones_t[:],
                        rhs=xts[c][:, kt, :],
                        start=(kt == 0),
                        stop=(kt == KT - 1),
                    )
                for kt in range(KT):
                    nc.tensor.matmul(
                        Sxx[:],
                        lhsT=ones_t[:],
                        rhs=sq[:, kt, :],
                        start=(kt == 0),
                        stop=(kt == KT - 1),
                    )
                # row math: -mu (bf16), mu^2, var, sigma (f32 + bf16)
                m2 = rows.tile([1, F], FP, tag="m2", name=f"m2{c}")
                nc.scalar.activation(
                    m2[:], Sx[:], func=AF.Square, bias=0.0, scale=1.0 / D
                )
                vt1 = rows.tile([1, F], FP, tag="vt1", name=f"vt1{c}")
                nc.vector.tensor_scalar(vt1[:], Sxx[:], 1.0 / D, None, ALU.mult)
                varx = rows.tile([1, F], FP, tag="varx", name=f"varx{c}")
                nc.vector.tensor_tensor(varx[:], vt1[:], m2[:], ALU.subtract)
                sgf = rows.tile([1, F], FP, tag="sgf", name=f"sgf{c}")
                nc.scalar.activation(
                    sgf[:], varx[:], func=AF.Sqrt, bias=eps_t[:], scale=1.0
                )
                # stack [-mu; sigma] on partitions 0/1 for one K=2 rank-1
                st2 = rows.tile([2, F], BF, tag="st2", name=f"st2{c}")
                nc.scalar.activation(
                    st2[0:1, :], Sx[:], func=AF.Copy, bias=0.0, scale=-1.0 / D
                )
                sgb = rows.tile([1, F], BF, tag="sgb", name=f"sgb{c}")
                nc.scalar.activation(
                    sgb[:], varx[:], func=AF.Sqrt, bias=eps_t[:], scale=1.0
                )
                nc.scalar.dma_start(st2[1:2, :], sgb[:])
                return st2, sgf

            def sig_transpose(c, sgf):
                # [1,512] sigma row -> r_sb[:, 4c:4c+4] columns via 4 tiny
                # PE transposes (rhs = 1x1 identity) + one PSUM reciprocal.
                pt = ptp.tile([P, CH], FP, tag="tp", name=f"sigT{c}")
                for u in range(CH):
                    nc.tensor.matmul(
                        pt[:, u : u + 1],
                        lhsT=sgf[0:1, u * P : (u + 1) * P],
                        rhs=idf[0:1, :],
                        is_transpose=True,
                        skip_group_check=True,
                        tile_position=(0, 0),
                    )
                nc.vector.reciprocal(r_sb[:, 4 * c : 4 * c + 4], pt[:])

            def q_chunk(c, st2):
                for e in range(ET):
                    ps = pmm.tile([P, F], FP, tag="ps", name=f"q{c}_{e}")
                    for kt in range(KT):
                        nc.tensor.matmul(
                            ps[:],
                            lhsT=wq_t[:, kt, e * P : (e + 1) * P],
                            rhs=xts[c][:, kt, :],
                            start=(kt == 0),
                            stop=False,
                        )
                    # K=2 rank-1: - mu (x) wqsum  +  sigma (x) q0
                    nc.tensor.matmul(
                        ps[:],
                        lhsT=wqq_t[0:2, e * P : (e + 1) * P],
                        rhs=st2[0:2, :],
                        start=False,
                        stop=True,
                    )
                    dst = qT[:, e, c * F : (c + 1) * F]
                    if e % 2 == 0:
                        nc.vector.tensor_copy(dst, ps[:])
                    else:
                        nc.scalar.copy(dst, ps[:])

            # ---- feed: PE order k0 s0 k1 T0 q0 s1 k2 T1 q1 s2 k3 T2 q2 s3 T3 q3
            k_chunk(0)
            nc.sync.dma_start(w2_t[:], w2.rearrange("(p kt) d -> p kt d", kt=KT))
            nc.scalar.dma_start(wq_t[:], wq.rearrange("(p kt) e -> p kt e", kt=KT))
            nc.scalar.dma_start(wqq_t[:], wqq)
            sta = stats_chunk(0)
            k_chunk(1)
            sig_transpose(0, sta[1])
            q_chunk(0, sta[0])
            stb = stats_chunk(1)
            k_chunk(2)
            sig_transpose(1, stb[1])
            q_chunk(1, stb[0])
            stc = stats_chunk(2)
            k_chunk(3)
            sig_transpose(2, stc[1])
            q_chunk(2, stc[0])
            std = stats_chunk(3)
            sig_transpose(3, std[1])
            q_chunk(3, std[0])

            # ---------------- sim, exp (+z via accum), colsum, Y -------------
            cs_all = pcs.tile([P, F], FP)
            nc.vector.memset(cs_all[:], 0.0)
            ex_hist: list = [None, None]
            zrb_hist: list = [None, None]

            def colsum_mms(it):
                ex_t = ex_hist[it % 2]
                zrb_t = zrb_hist[it % 2]
                for jc in range(JC):
                    nc.tensor.matmul(
                        cs_all[32 * jc : 32 * jc + 1, :],
                        lhsT=zrb_t[:],
                        rhs=ex_t[:, jc * F : (jc + 1) * F],
                        start=(it == 0),
                        stop=(it == NT - 1),
                        skip_group_check=True,
                        tile_position=(0, 32 * jc),
                    )

            def sim_group(it, jc, ex, zpart):
                ps = pmm.tile([P, F], FP, tag="ps", name=f"sim{it}_{jc}")
                for et in range(ET):
                    nc.tensor.matmul(
                        ps[:],
                        lhsT=qT[:, et, it * P : (it + 1) * P],
                        rhs=kT[:, et, jc * F : (jc + 1) * F],
                        start=(et == 0),
                        stop=(et == ET - 1),
                    )
                nc.scalar.activation(
                    ex[:, jc * F : (jc + 1) * F],
                    ps[:],
                    func=AF.Exp,
                    bias=0.0,
                    scale=r_sb[:, it : it + 1],
                    accum_out=zpart[:, jc : jc + 1],
                )

            def y_group(g):
                c, mb, dh = g // 8, (g % 8) // 2, g % 2
                jt = 4 * c + mb
                psn = pyy.tile([P, F], FP, tag="py", name=f"y{g}")
                for kt in range(KT):
                    nc.tensor.matmul(
                        psn[:],
                        lhsT=mts[c][:, kt, mb * P : (mb + 1) * P],
                        rhs=w2_t[:, kt, dh * F : (dh + 1) * F],
                        start=(kt == 0),
                        stop=(kt == KT - 1),
                    )
                nc.vector.tensor_copy(Y[:, jt, dh * F : (dh + 1) * F], psn[:])

            for it in range(NT):
                ex = expp.tile([P, M], BF, tag="ex", name=f"ex{it}")
                zpart = zsp.tile([P, JC], FP, tag="zpt", name=f"zpt{it}")
                sim_group(it, 0, ex, zpart)
                sim_group(it, 1, ex, zpart)
                y_group(2 * it)
                sim_group(it, 2, ex, zpart)
                if it > 0:
                    colsum_mms(it - 1)
                sim_group(it, 3, ex, zpart)
                y_group(2 * it + 1)
                z = zsp.tile([P, 1], FP, tag="z", name=f"z{it}")
                nc.vector.tensor_reduce(z[:], zpart[:], axis=AX.X, op=ALU.add)
                zr = zsp.tile([P, 1], FP, tag="zr", name=f"zr{it}")
                nc.vector.reciprocal(zr[:], z[:])
                zrb = zrbp.tile([P, 1], BF, tag="zrb", name=f"zrb{it}")
                nc.vector.tensor_copy(zrb[:], zr[:])
                ex_hist[it % 2] = ex
                zrb_hist[it % 2] = zrb
            colsum_mms(NT - 1)

            # ---------------- tail -------------------------------------------
            # per jc-chunk: 1-row csum evac -> 4 transposes -> colsb columns
            # -> 4 scales (DVE-heavy; ScalarE copies are 2.5x slower) -> 4
            # single-tile stores, triggers alternating gpsimd/sync queues.
            colT = ptp.tile([P, NT], FP, tag="tp", name="colT")
            for jc in range(JC):
                nc.scalar.copy(
                    csum_sb[32 * jc : 32 * jc + 1, :],
                    cs_all[32 * jc : 32 * jc + 1, :],
                )
                for bb in range(4):
                    jt = 4 * jc + bb
                    nc.tensor.matmul(
                        colT[:, jt : jt + 1],
                        lhsT=csum_sb[32 * jc : 32 * jc + 1, bb * P : (bb + 1) * P],
                        rhs=idf[32 * jc : 32 * jc + 1, :],
                        is_transpose=True,
                        skip_group_check=True,
                        tile_position=(32 * jc, 0),
                    )
                nc.vector.tensor_copy(
                    colsb[:, 4 * jc : 4 * jc + 4], colT[:, 4 * jc : 4 * jc + 4]
                )
                for bb in range(4):
                    jt = 4 * jc + bb
                    ob = obuf.tile([P, D], BF, tag="ob", name=f"ob{jt}")
                    csl = colsb[:, jt : jt + 1]
                    if jt % 4 == 1:
                        nc.scalar.mul(ob[:], Y[:, jt, :], csl)
                    else:
                        nc.vector.tensor_scalar_mul(ob[:], Y[:, jt, :], csl)
                    q = nc.gpsimd if jt % 2 == 0 else nc.sync
                    q.dma_start(ov[:, jt, :], ob[:])

    nc.compile()
    return nc


_NC_CACHE = None


def _get_nc():
    global _NC_CACHE
    if _NC_CACHE is None:
        _NC_CACHE = _build()
    return _NC_CACHE


BF_NP = ml_dtypes.bfloat16


def _prep(inputs):
    ln_w = np.asarray(inputs["ln_w"], dtype=np.float32)
    ln_b = np.asarray(inputs["ln_b"], dtype=np.float32)
    Wq = np.asarray(inputs["Wq"], dtype=np.float32)
    Wkv = np.asarray(inputs["Wkv"], dtype=np.float32)
    Wout = np.asarray(inputs["Wout"], dtype=np.float32)

    def permute_rows(w):  # row (kt*P + p) -> row (p*KT + kt) for big packets
        ct = w.shape[0] // P
        return np.ascontiguousarray(
            w.reshape(ct, P, w.shape[1]).transpose(1, 0, 2).reshape(w.shape)
        )

    wq_f = Wq * (SCALE * ln_w)[:, None]
    wq_h = permute_rows(wq_f.astype(BF_NP))
    wk_h = permute_rows(np.ascontiguousarray(Wkv[:, :E]).astype(BF_NP))
    w2_h = permute_rows((Wkv[:, E:] @ Wout).astype(BF_NP))
    wqq_h = np.ascontiguousarray(
        np.stack([wq_f.sum(0), SCALE * (ln_b @ Wq)]).astype(BF_NP)
    )

    def t_chunks(a):  # [2048, 1024] -> [(c p kt), i'] = [4096, 512]
        at = np.ascontiguousarray(a.astype(BF_NP).T)          # [D, n]
        return np.ascontiguousarray(
            at.reshape(KT, P, CH, F).transpose(2, 1, 0, 3).reshape(CH * D, F)
        )

    xs = np.asarray(inputs["x"], dtype=np.float32)
    ms = np.asarray(inputs["media"], dtype=np.float32)
    shared = {"wq": wq_h, "wk": wk_h, "w2": w2_h, "wqq": wqq_h}
    return [
        dict(shared, xt=t_chunks(xs[b]), mt=t_chunks(ms[b])) for b in range(B)
    ]


def _unscramble(o):  # [2048, 1024] HBM rows p*16+jt -> position rows jt*128+p
    return np.ascontiguousarray(
        o.reshape(P, NT, D).transpose(1, 0, 2).reshape(M, D)
    ).astype(np.float32)


def _run(inputs, trace=False, **kw):
    nc = _get_nc()
    in_maps = _prep(inputs)
    res = run_bass_kernel_spmd(nc, in_maps, core_ids=list(range(B)), trace=trace, **kw)
    out = np.stack(
        [_unscramble(res.results[b]["out"]) for b in range(B)], axis=0
    )
    return out, res


def kernel(**inputs) -> np.ndarray:
    out, _ = _run(inputs, trace=False)
    return out


# revision 15
# speedup vs baseline: 1.2388x; 1.0056x over previous
"""Trainium2 Bass kernel for nn_CrossAttention_47004122087816.

Math (faithful to the reference's "buggy einsum"):
    xn   = LayerNorm(x); xnb = xn * ln_w + ln_b
    q    = (xnb @ Wq) * SCALE            [n, E]
    k, v = split(media @ Wkv)            [m, E] each
    sim  = q @ k^T                       [n, m]
    colsum[j] = sum_i softmax(sim, -1)[i, j]
    out[j, :] = colsum[j] * (v @ Wout)[j, :]

Sharding: pure data-parallel - batch b=8 over 8 NeuronCores.

v2 redesign (vs the DMA-transpose baseline):
 - x and media are transposed on the HOST (layout-only prep, like the
   existing weight-row permutation), so the device loads land directly in
   the [D-part, rows-free] layout every matmul wants.  This removes all 32
   on-chip dma_start_transpose ops (~8MB of DMA traffic and the 35us
   startup serialization they caused).
 - LayerNorm is restructured to work in the transposed layout:
     q_i = r_i * (x_i @ wq' - mu_i * colsum(wq') + sigma_i * q0)
   with wq' = Wq * ln_w * SCALE, q0 = SCALE * ln_b @ Wq.  Sx and Sxx come
   from ones-vector matmuls (cheap M=1 PE work), the rank-1 corrections are
   K=1 matmuls accumulated into the q PSUM groups, and the final per-row
   scale r_i is folded into the sim-phase Exp activation's per-partition
   `scale` operand - zero extra elementwise passes over q.
 - v is never materialized: W2 = Wkv_v @ Wout is folded on the host, and
   Y = media @ W2 is computed directly (same FLOPs, one less PSUM
   evacuation pass and 2MB less SBUF).
 - sigma rows are flipped to per-partition columns with 16 tiny PE
   transposes (rhs = 1x1 identity); same trick turns the colsum PSUM rows
   into per-partition scalars in the tail, replacing 16 single-column
   scatter DMAs.
 - fp8 was evaluated (DoubleRow would halve PE time) and rejected: exp()
   amplification puts even k-only fp8 at ~2e-2 rel err, the whole gate.
 - Tail: colsum transposes -> one [128,16] copy -> 16 scaled copies
   (DVE/ScalarE alternating) -> paired 4KB-descriptor SWDGE stores.
"""

import sys

for _p in ("/opt/trn_rl_repo",):
    if _p not in sys.path:
        sys.path.insert(0, _p)

import numpy as np
import ml_dtypes

import concourse.bass as bass  # noqa: F401
import concourse.tile as tile
from concourse import bacc, mybir
from concourse.bass_utils import run_bass_kernel_spmd

B = 8
N = 2048          # x rows per batch element
M = 2048          # media rows per batch element
D = 1024          # model dim
E = 512           # inner dim
P = 128           # partitions
F = 512           # one PSUM bank of fp32
KT = D // P       # 8  contraction tiles over model dim
ET = E // P       # 4  contraction tiles over inner dim
NT = N // P       # 16 row tiles (positions)
JC = M // F       # 4  column chunks of 512
CH = N // F       # 4  position chunks of 512
SCALE = 64 ** -0.5
EPS = 1e-5

FP = mybir.dt.float32
BF = mybir.dt.bfloat16

AF = mybir.ActivationFunctionType
ALU = mybir.AluOpType
AX = mybir.AxisListType


def _build():
    nc = bacc.Bacc("TRN2", target_bir_lowering=False, debug=False, num_devices=B)

    # host layouts (see _run): xt/mt row (c*1024 + p*8 + kt) col i' holds
    # x[c*512 + i', kt*128 + p] -> per-partition 8KB contiguous loads.
    xt = nc.dram_tensor("xt", [CH * D, F], BF, kind="ExternalInput").ap()
    mt = nc.dram_tensor("mt", [CH * D, F], BF, kind="ExternalInput").ap()
    wq = nc.dram_tensor("wq", [D, E], BF, kind="ExternalInput").ap()
    wk = nc.dram_tensor("wk", [D, E], BF, kind="ExternalInput").ap()
    w2 = nc.dram_tensor("w2", [D, D], BF, kind="ExternalInput").ap()
    wqq = nc.dram_tensor("wqq", [2, E], BF, kind="ExternalInput").ap()
    out = nc.dram_tensor("out", [M, D], BF, kind="ExternalOutput").ap()

    xtv = xt.rearrange("(c p kt) i -> c p kt i", p=P, kt=KT)
    mtv = mt.rearrange("(c p kt) i -> c p kt i", p=P, kt=KT)
    # store tile jt partition p -> HBM row p*16+jt (host unscrambles);
    # paired stores give 4KB contiguous per-partition descriptors.
    ov = out.rearrange("(p t) d -> p t d", t=NT)

    with tile.TileContext(nc) as tc:
        from contextlib import ExitStack

        with ExitStack() as ctx:
            consts = ctx.enter_context(tc.tile_pool(name="consts", bufs=1))
            acts = ctx.enter_context(tc.tile_pool(name="acts", bufs=1))
            mtp = ctx.enter_context(tc.tile_pool(name="mtp", bufs=4))
            xtp = ctx.enter_context(tc.tile_pool(name="xtp", bufs=3))
            sqp = ctx.enter_context(tc.tile_pool(name="sqp", bufs=1))
            rows = ctx.enter_context(tc.tile_pool(name="rows", bufs=2))
            expp = ctx.enter_context(tc.tile_pool(name="expp", bufs=2))
            zsp = ctx.enter_context(tc.tile_pool(name="zsp", bufs=2))
            zrbp = ctx.enter_context(tc.tile_pool(name="zrbp", bufs=2))
            obuf = ctx.enter_context(tc.tile_pool(name="obuf", bufs=6))
            pmm = ctx.enter_context(tc.tile_pool(name="pmm", bufs=3, space="PSUM"))
            pyy = ctx.enter_context(tc.tile_pool(name="pyy", bufs=1, space="PSUM"))
            pst = ctx.enter_context(tc.tile_pool(name="pst", bufs=1, space="PSUM"))
            ptp = ctx.enter_context(tc.tile_pool(name="ptp", bufs=1, space="PSUM"))
            pcs = ctx.enter_context(tc.tile_pool(name="pcs", bufs=1, space="PSUM"))

            wq_t = consts.tile([P, KT, E], BF)
            wk_t = consts.tile([P, KT, E], BF)
            w2_t = consts.tile([P, KT, D], BF)
            wqq_t = consts.tile([2, E], BF)
            wdum = consts.tile([P, F], BF)
            ones_t = consts.tile([P, 1], BF)
            idf = consts.tile([P, 1], FP)     # 1x1 identity slices for transposes
            eps_t = consts.tile([1, 1], FP)
            gdum = consts.tile([1, 4], BF)    # w2 DMA gate (reads wk_t)
            r_sb = consts.tile([P, NT], FP)   # 1/sigma per position column
            colsb = consts.tile([P, NT], FP)
            csum_sb = consts.tile([P, F], FP)

            kT = acts.tile([P, ET, M], BF)
            qT = acts.tile([P, ET, N], BF)
            Y = acts.tile([P, NT, D], BF)

            nc.vector.memset(ones_t[:], 1.0)
            nc.vector.memset(idf[:], 1.0)
            nc.vector.memset(eps_t[:], EPS)
            nc.vector.memset(wdum[:], 0.125)
            # PE p-state warm-up during the initial DMA window
            for wdi in range(5):
                pd = pmm.tile([P, F], FP, tag="ps", name=f"warm{wdi}")
                nc.tensor.matmul(
                    pd[:], lhsT=wdum[:, 0:P], rhs=wdum[:], start=True, stop=True
                )

            # ---------------- bulk loads -------------------------------------
            mts: list = []
            xts: list = []

            def load_m(c):
                t = mtp.tile([P, KT, F], BF, tag="mt", name=f"mt{c}")
                nc.gpsimd.dma_start(t[:], mtv[c])
                mts.append(t)

            def load_x(c):
                t = xtp.tile([P, KT, F], BF, tag="xt", name=f"xt{c}")
                nc.gpsimd.dma_start(t[:], xtv[c])
                xts.append(t)

            # SWDGE in need-order; wk/wq/wqq on scalar HWDGE; w2 on sync,
            # emitted after k_chunk(0) so k0's queue-semaphore wait
            # does not cover it.
            wkv = wk.rearrange("(p kt) e -> p kt e", kt=KT)
            nc.scalar.dma_start(wk_t[:, 0 : KT // 2, :], wkv[:, 0 : KT // 2, :])
            nc.scalar.dma_start(wk_t[:, KT // 2 :, :], wkv[:, KT // 2 :, :])
            t0 = mtp.tile([P, KT, F], BF, tag="mt", name="mt0")
            nc.gpsimd.dma_start(t0[:, 0 : KT // 2, :], mtv[0][:, 0 : KT // 2, :])
            nc.gpsimd.dma_start(t0[:, KT // 2 :, :], mtv[0][:, KT // 2 :, :])
            mts.append(t0)
            load_x(0)
            load_m(1)
            load_x(1)
            load_m(2)
            load_x(2)
            load_m(3)
            load_x(3)
            # ---------------- feed helpers -----------------------------------
            def k_chunk(c):
                for e in range(ET):
                    ps = pmm.tile([P, F], FP, tag="ps", name=f"k{c}_{e}")
                    for kt in range(KT):
                        nc.tensor.matmul(
                            ps[:],
                            lhsT=wk_t[:, kt, e * P : (e + 1) * P],
                            rhs=mts[c][:, kt, :],
                            start=(kt == 0),
                            stop=(kt == KT - 1),
                        )
                    dst = kT[:, e, c * F : (c + 1) * F]
                    if e % 2 == 0:
                        nc.scalar.copy(dst, ps[:])
                    else:
                        nc.vector.tensor_copy(dst, ps[:])

            def stats_chunk(c):
                sq = sqp.tile([P, KT, F], BF, tag="sq", name=f"sq{c}")
                nc.vector.tensor_tensor(sq[:], xts[c][:], xts[c][:], ALU.mult)
                Sx = pst.tile([1, F], FP, tag="sx", name=f"sx{c}")
                Sxx = pst.tile([1, F], FP, tag="sxx", name=f"sxx{c}")
                for kt in range(KT):
                    nc.tensor.matmul(
                        Sx[:],
                        lhsT=ones_t[:],
                        rhs=xts[c][:, kt, :],
                        start=(kt == 0),
                        stop=(kt == KT - 1),
                    )
                for kt in range(KT):
                    nc.tensor.matmul(
                        Sxx[:],
                        lhsT=ones_t[:],
                        rhs=sq[:, kt, :],
                        start=(kt == 0),
                        stop=(kt == KT - 1),
                    )
                # row math: -mu (bf16), mu^2, var, sigma (f32 + bf16)
                m2 = rows.tile([1, F], FP, tag="m2", name=f"m2{c}")
                nc.scalar.activation(
                    m2[:], Sx[:], func=AF.Square, bias=0.0, scale=1.0 / D
                )
                vt1 = rows.tile([1, F], FP, tag="vt1", name=f"vt1{c}")
                nc.vector.tensor_scalar(vt1[:], Sxx[:], 1.0 / D, None, ALU.mult)
                varx = rows.tile([1, F], FP, tag="varx", name=f"varx{c}")
                nc.vector.tensor_tensor(varx[:], vt1[:], m2[:], ALU.subtract)
                sgf = rows.tile([1, F], FP, tag="sgf", name=f"sgf{c}")
                nc.scalar.activation(
                    sgf[:], varx[:], func=AF.Sqrt, bias=eps_t[:], scale=1.0
                )
                # stack [-mu; sigma] on partitions 0/1 for one K=2 rank-1
                st2 = rows.tile([2, F], BF, tag="st2", name=f"st2{c}")
                nc.scalar.activation(
                    st2[0:1, :], Sx[:], func=AF.Copy, bias=0.0, scale=-1.0 / D
                )
                sgb = rows.tile([1, F], BF, tag="sgb", name=f"sgb{c}")
                nc.scalar.activation(
                    sgb[:], varx[:], func=AF.Sqrt, bias=eps_t[:], scale=1.0
                )
                nc.scalar.dma_start(st2[1:2, :], sgb[:])
                return st2, sgf

            def sig_transpose(c, sgf):
                # [1,512] sigma row -> r_sb[:, 4c:4c+4] columns via 4 tiny
                # PE transposes (rhs = 1x1 identity) + one PSUM reciprocal.
                pt = ptp.tile([P, CH], FP, tag="tp", name=f"sigT{c}")
                for u in range(CH):
                    nc.tensor.matmul(
                        pt[:, u : u + 1],
                        lhsT=sgf[0:1, u * P : (u + 1) * P],
                        rhs=idf[0:1, :],
                        is_transpose=True,
                        skip_group_check=True,
                        tile_position=(0, 0),
                    )
                nc.vector.reciprocal(r_sb[:, 4 * c : 4 * c + 4], pt[:])

            def q_chunk(c, st2):
                for e in range(ET):
                    ps = pmm.tile([P, F], FP, tag="ps", name=f"q{c}_{e}")
                    for kt in range(KT):
                        nc.tensor.matmul(
                            ps[:],
                            lhsT=wq_t[:, kt, e * P : (e + 1) * P],
                            rhs=xts[c][:, kt, :],
                            start=(kt == 0),
                            stop=False,
                        )
                    # K=2 rank-1: - mu (x) wqsum  +  sigma (x) q0
                    nc.tensor.matmul(
                        ps[:],
                        lhsT=wqq_t[0:2, e * P : (e + 1) * P],
                        rhs=st2[0:2, :],
                        start=False,
                        stop=True,
                    )
                    dst = qT[:, e, c * F : (c + 1) * F]
                    if e % 2 == 0:
                        nc.vector.tensor_copy(dst, ps[:])
                    else:
                        nc.scalar.copy(dst, ps[:])

            # ---- feed: PE order k0 s0 k1 T0 q0 s1 k2 T1 q1 s2 k3 T2 q2 s3 T3 q3
            k_chunk(0)
            # the DMA engines round-robin across active queues, so w2 must
            # not stream while k0 still waits on wk/mt0: gate it behind a
            # 4-byte DMA that reads both wk_t halves.
            nc.sync.dma_start(gdum[:], wk_t[0:1, KT // 2 - 1 : KT // 2 + 1, 0:2])
            nc.sync.dma_start(w2_t[:], w2.rearrange("(p kt) d -> p kt d", kt=KT))
            nc.scalar.dma_start(wq_t[:], wq.rearrange("(p kt) e -> p kt e", kt=KT))
            nc.scalar.dma_start(wqq_t[:], wqq)
            sta = stats_chunk(0)
            k_chunk(1)
            sig_transpose(0, sta[1])
            q_chunk(0, sta[0])
            stb = stats_chunk(1)
            k_chunk(2)
            sig_transpose(1, stb[1])
            q_chunk(1, stb[0])
            stc = stats_chunk(2)
            k_chunk(3)
            sig_transpose(2, stc[1])
            q_chunk(2, stc[0])
            std = stats_chunk(3)
            sig_transpose(3, std[1])
            q_chunk(3, std[0])

            # ---------------- sim, exp (+z via accum), colsum, Y -------------
            cs_all = pcs.tile([P, F], FP)
            nc.vector.memset(cs_all[:], 0.0)
            ex_hist: list = [None, None]
            zrb_hist: list = [None, None]

            def colsum_mms(it):
                ex_t = ex_hist[it % 2]
                zrb_t = zrb_hist[it % 2]
                for jc in range(JC):
                    nc.tensor.matmul(
                        cs_all[32 * jc : 32 * jc + 1, :],
                        lhsT=zrb_t[:],
                        rhs=ex_t[:, jc * F : (jc + 1) * F],
                        start=(it == 0),
                        stop=(it == NT - 1),
                        skip_group_check=True,
                        tile_position=(0, 32 * jc),
                    )

            def sim_group(it, jc, ex, zpart):
                ps = pmm.tile([P, F], FP, tag="ps", name=f"sim{it}_{jc}")
                for et in range(ET):
                    nc.tensor.matmul(
                        ps[:],
                        lhsT=qT[:, et, it * P : (it + 1) * P],
                        rhs=kT[:, et, jc * F : (jc + 1) * F],
                        start=(et == 0),
                        stop=(et == ET - 1),
                    )
                nc.scalar.activation(
                    ex[:, jc * F : (jc + 1) * F],
                    ps[:],
                    func=AF.Exp,
                    bias=0.0,
                    scale=r_sb[:, it : it + 1],
                    accum_out=zpart[:, jc : jc + 1],
                )

            def y_group(g):
                c, mb, dh = g // 8, (g % 8) // 2, g % 2
                jt = 4 * c + mb
                psn = pyy.tile([P, F], FP, tag="py", name=f"y{g}")
                for kt in range(KT):
                    nc.tensor.matmul(
                        psn[:],
                        lhsT=mts[c][:, kt, mb * P : (mb + 1) * P],
                        rhs=w2_t[:, kt, dh * F : (dh + 1) * F],
                        start=(kt == 0),
                        stop=(kt == KT - 1),
                    )
                nc.vector.tensor_copy(Y[:, jt, dh * F : (dh + 1) * F], psn[:])

            for it in range(NT):
                ex = expp.tile([P, M], BF, tag="ex", name=f"ex{it}")
                zpart = zsp.tile([P, JC], FP, tag="zpt", name=f"zpt{it}")
                sim_group(it, 0, ex, zpart)
                sim_group(it, 1, ex, zpart)
                y_group(2 * it)
                sim_group(it, 2, ex, zpart)
                if it > 0:
                    colsum_mms(it - 1)
                sim_group(it, 3, ex, zpart)
                y_group(2 * it + 1)
                z = zsp.tile([P, 1], FP, tag="z", name=f"z{it}")
                nc.vector.tensor_reduce(z[:], zpart[:], axis=AX.X, op=ALU.add)
                zr = zsp.tile([P, 1], FP, tag="zr", name=f"zr{it}")
                nc.vector.reciprocal(zr[:], z[:])
                zrb = zrbp.tile([P, 1], BF, tag="zrb", name=f"zrb{it}")
                nc.vector.tensor_copy(zrb[:], zr[:])
                ex_hist[it % 2] = ex
                zrb_hist[it % 2] = zrb
            colsum_mms(NT - 1)

            # ---------------- tail -------------------------------------------
            # per jc-chunk: 1-row csum evac -> 4 transposes -> colsb columns
            # -> 4 scales (DVE-heavy; ScalarE copies are 2.5x slower) -> 4
            # single-tile stores, triggers alternating gpsimd/sync queues.
            colT = ptp.tile([P, NT], FP, tag="tp", name="colT")
            for jc in range(JC):
                src = cs_all[32 * jc : 32 * jc + 1, :]
                dst = csum_sb[32 * jc : 32 * jc + 1, :]
                if jc % 2 == 0:
                    nc.vector.tensor_copy(dst, src)
                else:
                    nc.scalar.copy(dst, src)
            for jc in range(JC):
                for bb in range(4):
                    jt = 4 * jc + bb
                    nc.tensor.matmul(
                        colT[:, jt : jt + 1],
                        lhsT=csum_sb[32 * jc : 32 * jc + 1, bb * P : (bb + 1) * P],
                        rhs=idf[32 * jc : 32 * jc + 1, :],
                        is_transpose=True,
                        skip_group_check=True,
                        tile_position=(32 * jc, 0),
                    )
                nc.vector.tensor_copy(
                    colsb[:, 4 * jc : 4 * jc + 4], colT[:, 4 * jc : 4 * jc + 4]
                )
                for bb in range(4):
                    jt = 4 * jc + bb
                    ob = obuf.tile([P, D], BF, tag="ob", name=f"ob{jt}")
                    csl = colsb[:, jt : jt + 1]
                    if jt % 4 == 1:
                        nc.scalar.mul(ob[:], Y[:, jt, :], csl)
                    else:
                        nc.vector.tensor_scalar_mul(ob[:], Y[:, jt, :], csl)
                    q = nc.gpsimd if jt % 2 == 0 else nc.sync
                    q.dma_start(ov[:, jt, :], ob[:])

    nc.compile()
    return nc


_NC_CACHE = None


def _get_nc():
    global _NC_CACHE
    if _NC_CACHE is None:
        _NC_CACHE = _build()
    return _NC_CACHE


BF_NP = ml_dtypes.bfloat16


def _prep(inputs):
    ln_w = np.asarray(inputs["ln_w"], dtype=np.float32)
    ln_b = np.asarray(inputs["ln_b"], dtype=np.float32)
    Wq = np.asarray(inputs["Wq"], dtype=np.float32)
    Wkv = np.asarray(inputs["Wkv"], dtype=np.float32)
    Wout = np.asarray(inputs["Wout"], dtype=np.float32)

    def permute_rows(w):  # row (kt*P + p) -> row (p*KT + kt) for big packets
        ct = w.shape[0] // P
        return np.ascontiguousarray(
            w.reshape(ct, P, w.shape[1]).transpose(1, 0, 2).reshape(w.shape)
        )

    wq_f = Wq * (SCALE * ln_w)[:, None]
    wq_h = permute_rows(wq_f.astype(BF_NP))
    wk_h = permute_rows(np.ascontiguousarray(Wkv[:, :E]).astype(BF_NP))
    w2_h = permute_rows((Wkv[:, E:] @ Wout).astype(BF_NP))
    wqq_h = np.ascontiguousarray(
        np.stack([wq_f.sum(0), SCALE * (ln_b @ Wq)]).astype(BF_NP)
    )

    def t_chunks(a):  # [2048, 1024] -> [(c p kt), i'] = [4096, 512]
        at = np.ascontiguousarray(a.astype(BF_NP).T)          # [D, n]
        return np.ascontiguousarray(
            at.reshape(KT, P, CH, F).transpose(2, 1, 0, 3).reshape(CH * D, F)
        )

    xs = np.asarray(inputs["x"], dtype=np.float32)
    ms = np.asarray(inputs["media"], dtype=np.float32)
    shared = {"wq": wq_h, "wk": wk_h, "w2": w2_h, "wqq": wqq_h}
    return [
        dict(shared, xt=t_chunks(xs[b]), mt=t_chunks(ms[b])) for b in range(B)
    ]


def _unscramble(o):  # [2048, 1024] HBM rows p*16+jt -> position rows jt*128+p
    return np.ascontiguousarray(
        o.reshape(P, NT, D).transpose(1, 0, 2).reshape(M, D)
    ).astype(np.float32)


def _run(inputs, trace=False, **kw):
    nc = _get_nc()
    in_maps = _prep(inputs)
    res = run_bass_kernel_spmd(nc, in_maps, core_ids=list(range(B)), trace=trace, **kw)
    out = np.stack(
        [_unscramble(res.results[b]["out"]) for b in range(B)], axis=0
    )
    return out, res


def kernel(**inputs) -> np.ndarray:
    out, _ = _run(inputs, trace=False)
    return out


# revision 17
# speedup vs baseline: 1.2445x; 1.0046x over previous
"""Trainium2 Bass kernel for nn_CrossAttention_47004122087816.

Math (faithful to the reference's "buggy einsum"):
    xn   = LayerNorm(x); xnb = xn * ln_w + ln_b
    q    = (xnb @ Wq) * SCALE            [n, E]
    k, v = split(media @ Wkv)            [m, E] each
    sim  = q @ k^T                       [n, m]
    colsum[j] = sum_i softmax(sim, -1)[i, j]
    out[j, :] = colsum[j] * (v @ Wout)[j, :]

Sharding: pure data-parallel - batch b=8 over 8 NeuronCores.

v2 redesign (vs the DMA-transpose baseline):
 - x and media are transposed on the HOST (layout-only prep, like the
   existing weight-row permutation), so the device loads land directly in
   the [D-part, rows-free] layout every matmul wants.  This removes all 32
   on-chip dma_start_transpose ops (~8MB of DMA traffic and the 35us
   startup serialization they caused).
 - LayerNorm is restructured to work in the transposed layout:
     q_i = r_i * (x_i @ wq' - mu_i * colsum(wq') + sigma_i * q0)
   with wq' = Wq * ln_w * SCALE, q0 = SCALE * ln_b @ Wq.  Sx and Sxx come
   from ones-vector matmuls (cheap M=1 PE work), the rank-1 corrections are
   K=1 matmuls accumulated into the q PSUM groups, and the final per-row
   scale r_i is folded into the sim-phase Exp activation's per-partition
   `scale` operand - zero extra elementwise passes over q.
 - v is never materialized: W2 = Wkv_v @ Wout is folded on the host, and
   Y = media @ W2 is computed directly (same FLOPs, one less PSUM
   evacuation pass and 2MB less SBUF).
 - sigma rows are flipped to per-partition columns with 16 tiny PE
   transposes (rhs = 1x1 identity); same trick turns the colsum PSUM rows
   into per-partition scalars in the tail, replacing 16 single-column
   scatter DMAs.
 - fp8 was evaluated (DoubleRow would halve PE time) and rejected: exp()
   amplification puts even k-only fp8 at ~2e-2 rel err, the whole gate.
 - Tail: colsum transposes -> one [128,16] copy -> 16 scaled copies
   (DVE/ScalarE alternating) -> paired 4KB-descriptor SWDGE stores.
"""

import sys

for _p in ("/opt/trn_rl_repo",):
    if _p not in sys.path:
        sys.path.insert(0, _p)

import numpy as np
import ml_dtypes

import concourse.bass as bass  # noqa: F401
import concourse.tile as tile
from concourse import bacc, mybir
from concourse.bass_utils import run_bass_kernel_spmd

B = 8
N = 2048          # x rows per batch element
M = 2048          # media rows per batch element
D = 1024          # model dim
E = 512           # inner dim
P = 128           # partitions
F = 512           # one PSUM bank of fp32
KT = D // P       # 8  contraction tiles over model dim
ET = E // P       # 4  contraction tiles over inner dim
NT = N // P       # 16 row tiles (positions)
JC = M // F       # 4  column chunks of 512
CH = N // F       # 4  position chunks of 512
SCALE = 64 ** -0.5
EPS = 1e-5

FP = mybir.dt.float32
BF = mybir.dt.bfloat16

AF = mybir.ActivationFunctionType
ALU = mybir.AluOpType
AX = mybir.AxisListType


def _build():
    nc = bacc.Bacc("TRN2", target_bir_lowering=False, debug=False, num_devices=B)

    # host layouts (see _run): xt/mt row (c*1024 + p*8 + kt) col i' holds
    # x[c*512 + i', kt*128 + p] -> per-partition 8KB contiguous loads.
    xt = nc.dram_tensor("xt", [CH * D, F], BF, kind="ExternalInput").ap()
    mt = nc.dram_tensor("mt", [CH * D, F], BF, kind="ExternalInput").ap()
    wq = nc.dram_tensor("wq", [D, E], BF, kind="ExternalInput").ap()
    wk = nc.dram_tensor("wk", [D, E], BF, kind="ExternalInput").ap()
    w2 = nc.dram_tensor("w2", [D, D], BF, kind="ExternalInput").ap()
    wqq = nc.dram_tensor("wqq", [2, E], BF, kind="ExternalInput").ap()
    out = nc.dram_tensor("out", [M, D], BF, kind="ExternalOutput").ap()

    xtv = xt.rearrange("(c p kt) i -> c p kt i", p=P, kt=KT)
    mtv = mt.rearrange("(c p kt) i -> c p kt i", p=P, kt=KT)
    # store tile jt partition p -> HBM row p*16+jt (host unscrambles);
    # paired stores give 4KB contiguous per-partition descriptors.
    ov = out.rearrange("(p t) d -> p t d", t=NT)

    with tile.TileContext(nc) as tc:
        from contextlib import ExitStack

        with ExitStack() as ctx:
            consts = ctx.enter_context(tc.tile_pool(name="consts", bufs=1))
            acts = ctx.enter_context(tc.tile_pool(name="acts", bufs=1))
            mtp = ctx.enter_context(tc.tile_pool(name="mtp", bufs=4))
            xtp = ctx.enter_context(tc.tile_pool(name="xtp", bufs=2))
            sqp = ctx.enter_context(tc.tile_pool(name="sqp", bufs=1))
            rows = ctx.enter_context(tc.tile_pool(name="rows", bufs=2))
            expp = ctx.enter_context(tc.tile_pool(name="expp", bufs=2))
            zsp = ctx.enter_context(tc.tile_pool(name="zsp", bufs=2))
            zrbp = ctx.enter_context(tc.tile_pool(name="zrbp", bufs=2))
            obuf = ctx.enter_context(tc.tile_pool(name="obuf", bufs=2))
            pmm = ctx.enter_context(tc.tile_pool(name="pmm", bufs=3, space="PSUM"))
            pyy = ctx.enter_context(tc.tile_pool(name="pyy", bufs=1, space="PSUM"))
            pst = ctx.enter_context(tc.tile_pool(name="pst", bufs=1, space="PSUM"))
            ptp = ctx.enter_context(tc.tile_pool(name="ptp", bufs=1, space="PSUM"))
            pcs = ctx.enter_context(tc.tile_pool(name="pcs", bufs=1, space="PSUM"))

            wq_t = consts.tile([P, KT, E], BF)
            wk_t = consts.tile([P, KT, E], BF)
            w2_t = consts.tile([P, KT, D], BF)
            wqq_t = consts.tile([2, E], BF)
            wdum = consts.tile([P, F], BF)
            ones_t = consts.tile([P, 1], BF)
            idf = consts.tile([P, 1], FP)     # 1x1 identity slices for transposes
            eps_t = consts.tile([1, 1], FP)
            r_sb = consts.tile([P, NT], FP)   # 1/sigma per position column
            colsb = consts.tile([P, NT], FP)
            csum_sb = consts.tile([P, F], FP)

            kT = acts.tile([P, ET, M], BF)
            qT = acts.tile([P, ET, N], BF)
            Y = acts.tile([P, NT, D], BF)

            nc.vector.memset(ones_t[:], 1.0)
            nc.vector.memset(idf[:], 1.0)
            nc.vector.memset(eps_t[:], EPS)
            nc.vector.memset(wdum[:], 0.125)
            # PE p-state warm-up during the initial DMA window
            for wdi in range(5):
                pd = pmm.tile([P, F], FP, tag="ps", name=f"warm{wdi}")
                nc.tensor.matmul(
                    pd[:], lhsT=wdum[:, 0:P], rhs=wdum[:], start=True, stop=True
                )

            # ---------------- bulk loads -------------------------------------
            mts: list = []
            xts: list = []

            def load_m(c):
                t = mtp.tile([P, KT, F], BF, tag="mt", name=f"mt{c}")
                nc.gpsimd.dma_start(t[:], mtv[c])
                mts.append(t)

            def load_x(c):
                t = xtp.tile([P, KT, F], BF, tag="xt", name=f"xt{c}")
                nc.gpsimd.dma_start(t[:], xtv[c])
                xts.append(t)

            # SWDGE in need-order; wk/wq/wqq on scalar HWDGE; w2 on sync,
            # emitted after k_chunk(0) so k0's queue-semaphore wait
            # does not cover it.
            wkv = wk.rearrange("(p kt) e -> p kt e", kt=KT)
            nc.scalar.dma_start(wk_t[:, 0 : KT // 2, :], wkv[:, 0 : KT // 2, :])
            nc.scalar.dma_start(wk_t[:, KT // 2 :, :], wkv[:, KT // 2 :, :])
            # mt0 on the sync HWDGE queue: it streams from ~8.6us while
            # the SWDGE ring is still spinning up (~12.7us).
            t0 = mtp.tile([P, KT, F], BF, tag="mt", name="mt0")
            nc.sync.dma_start(t0[:, 0 : KT // 2, :], mtv[0][:, 0 : KT // 2, :])
            nc.sync.dma_start(t0[:, KT // 2 :, :], mtv[0][:, KT // 2 :, :])
            mts.append(t0)
            load_x(0)
            load_m(1)
            load_x(1)
            load_m(2)
            load_x(2)
            load_m(3)
            load_x(3)
            # w2 last on SWDGE: streams only after all x/media loads, so it
            # never competes with feed-critical traffic; ready well before
            # the first y_group.
            nc.gpsimd.dma_start(
                w2_t[:], w2.rearrange("(p kt) d -> p kt d", kt=KT)
            )
            # ---------------- feed helpers -----------------------------------
            def k_chunk(c):
                for e in range(ET):
                    ps = pmm.tile([P, F], FP, tag="ps", name=f"k{c}_{e}")
                    for kt in range(KT):
                        nc.tensor.matmul(
                            ps[:],
                            lhsT=wk_t[:, kt, e * P : (e + 1) * P],
                            rhs=mts[c][:, kt, :],
                            start=(kt == 0),
                            stop=(kt == KT - 1),
                        )
                    dst = kT[:, e, c * F : (c + 1) * F]
                    if e % 2 == 0:
                        nc.scalar.copy(dst, ps[:])
                    else:
                        nc.vector.tensor_copy(dst, ps[:])

            def stats_chunk(c):
                sq = sqp.tile([P, KT, F], BF, tag="sq", name=f"sq{c}")
                nc.vector.tensor_tensor(sq[:], xts[c][:], xts[c][:], ALU.mult)
                Sx = pst.tile([1, F], FP, tag="sx", name=f"sx{c}")
                Sxx = pst.tile([1, F], FP, tag="sxx", name=f"sxx{c}")
                for kt in range(KT):
                    nc.tensor.matmul(
                        Sx[:],
                        lhsT=ones_t[:],
                        rhs=xts[c][:, kt, :],
                        start=(kt == 0),
                        stop=(kt == KT - 1),
                    )
                for kt in range(KT):
                    nc.tensor.matmul(
                        Sxx[:],
                        lhsT=ones_t[:],
                        rhs=sq[:, kt, :],
                        start=(kt == 0),
                        stop=(kt == KT - 1),
                    )
                # row math: -mu (bf16), mu^2, var, sigma (f32 + bf16)
                m2 = rows.tile([1, F], FP, tag="m2", name=f"m2{c}")
                nc.scalar.activation(
                    m2[:], Sx[:], func=AF.Square, bias=0.0, scale=1.0 / D
                )
                vt1 = rows.tile([1, F], FP, tag="vt1", name=f"vt1{c}")
                nc.vector.tensor_scalar(vt1[:], Sxx[:], 1.0 / D, None, ALU.mult)
                varx = rows.tile([1, F], FP, tag="varx", name=f"varx{c}")
                nc.vector.tensor_tensor(varx[:], vt1[:], m2[:], ALU.subtract)
                sgf = rows.tile([1, F], FP, tag="sgf", name=f"sgf{c}")
                nc.scalar.activation(
                    sgf[:], varx[:], func=AF.Sqrt, bias=eps_t[:], scale=1.0
                )
                # stack [-mu; sigma] on partitions 0/1 for one K=2 rank-1
                st2 = rows.tile([2, F], BF, tag="st2", name=f"st2{c}")
                nc.scalar.activation(
                    st2[0:1, :], Sx[:], func=AF.Copy, bias=0.0, scale=-1.0 / D
                )
                sgb = rows.tile([1, F], BF, tag="sgb", name=f"sgb{c}")
                nc.scalar.activation(
                    sgb[:], varx[:], func=AF.Sqrt, bias=eps_t[:], scale=1.0
                )
                nc.sync.dma_start(st2[1:2, :], sgb[:])
                return st2, sgf

            def sig_transpose(c, sgf):
                # [1,512] sigma row -> r_sb[:, 4c:4c+4] columns via 4 tiny
                # PE transposes (rhs = 1x1 identity) + one PSUM reciprocal.
                pt = ptp.tile([P, CH], FP, tag="tp", name=f"sigT{c}")
                for u in range(CH):
                    nc.tensor.matmul(
                        pt[:, u : u + 1],
                        lhsT=sgf[0:1, u * P : (u + 1) * P],
                        rhs=idf[0:1, :],
                        is_transpose=True,
                        skip_group_check=True,
                        tile_position=(0, 0),
                    )
                nc.vector.reciprocal(r_sb[:, 4 * c : 4 * c + 4], pt[:])

            def q_chunk(c, st2):
                for e in range(ET):
                    ps = pmm.tile([P, F], FP, tag="ps", name=f"q{c}_{e}")
                    for kt in range(KT):
                        nc.tensor.matmul(
                            ps[:],
                            lhsT=wq_t[:, kt, e * P : (e + 1) * P],
                            rhs=xts[c][:, kt, :],
                            start=(kt == 0),
                            stop=False,
                        )
                    # K=2 rank-1: - mu (x) wqsum  +  sigma (x) q0
                    nc.tensor.matmul(
                        ps[:],
                        lhsT=wqq_t[0:2, e * P : (e + 1) * P],
                        rhs=st2[0:2, :],
                        start=False,
                        stop=True,
                    )
                    dst = qT[:, e, c * F : (c + 1) * F]
                    if e % 2 == 0:
                        nc.vector.tensor_copy(dst, ps[:])
                    else:
                        nc.scalar.copy(dst, ps[:])

            # ---- feed: PE order k0 s0 k1 T0 q0 s1 k2 T1 q1 s2 k3 T2 q2 s3 T3 q3
            k_chunk(0)
            nc.scalar.dma_start(wq_t[:], wq.rearrange("(p kt) e -> p kt e", kt=KT))
            nc.scalar.dma_start(wqq_t[:], wqq)
            sta = stats_chunk(0)
            k_chunk(1)
            sig_transpose(0, sta[1])
            q_chunk(0, sta[0])
            stb = stats_chunk(1)
            k_chunk(2)
            sig_transpose(1, stb[1])
            q_chunk(1, stb[0])
            stc = stats_chunk(2)
            k_chunk(3)
            sig_transpose(2, stc[1])
            q_chunk(2, stc[0])
            std = stats_chunk(3)
            sig_transpose(3, std[1])
            q_chunk(3, std[0])

            # ---------------- sim, exp (+z via accum), colsum, Y -------------
            cs_all = pcs.tile([P, F], FP)
            nc.vector.memset(cs_all[:], 0.0)
            ex_hist: list = [None, None]
            zrb_hist: list = [None, None]

            def colsum_mms(it):
                ex_t = ex_hist[it % 2]
                zrb_t = zrb_hist[it % 2]
                for jc in range(JC):
                    nc.tensor.matmul(
                        cs_all[32 * jc : 32 * jc + 1, :],
                        lhsT=zrb_t[:],
                        rhs=ex_t[:, jc * F : (jc + 1) * F],
                        start=(it == 0),
                        stop=(it == NT - 1),
                        skip_group_check=True,
                        tile_position=(0, 32 * jc),
                    )

            def sim_group(it, jc, ex, zpart):
                ps = pmm.tile([P, F], FP, tag="ps", name=f"sim{it}_{jc}")
                for et in range(ET):
                    nc.tensor.matmul(
                        ps[:],
                        lhsT=qT[:, et, it * P : (it + 1) * P],
                        rhs=kT[:, et, jc * F : (jc + 1) * F],
                        start=(et == 0),
                        stop=(et == ET - 1),
                    )
                nc.scalar.activation(
                    ex[:, jc * F : (jc + 1) * F],
                    ps[:],
                    func=AF.Exp,
                    bias=0.0,
                    scale=r_sb[:, it : it + 1],
                    accum_out=zpart[:, jc : jc + 1],
                )

            def y_group(g):
                c, mb, dh = g // 8, (g % 8) // 2, g % 2
                jt = 4 * c + mb
                psn = pyy.tile([P, F], FP, tag="py", name=f"y{g}")
                for kt in range(KT):
                    nc.tensor.matmul(
                        psn[:],
                        lhsT=mts[c][:, kt, mb * P : (mb + 1) * P],
                        rhs=w2_t[:, kt, dh * F : (dh + 1) * F],
                        start=(kt == 0),
                        stop=(kt == KT - 1),
                    )
                nc.vector.tensor_copy(Y[:, jt, dh * F : (dh + 1) * F], psn[:])

            for it in range(NT):
                ex = expp.tile([P, M], BF, tag="ex", name=f"ex{it}")
                zpart = zsp.tile([P, JC], FP, tag="zpt", name=f"zpt{it}")
                sim_group(it, 0, ex, zpart)
                sim_group(it, 1, ex, zpart)
                y_group(2 * it)
                sim_group(it, 2, ex, zpart)
                if it > 0:
                    colsum_mms(it - 1)
                sim_group(it, 3, ex, zpart)
                y_group(2 * it + 1)
                z = zsp.tile([P, 1], FP, tag="z", name=f"z{it}")
                nc.vector.tensor_reduce(z[:], zpart[:], axis=AX.X, op=ALU.add)
                zr = zsp.tile([P, 1], FP, tag="zr", name=f"zr{it}")
                nc.vector.reciprocal(zr[:], z[:])
                zrb = zrbp.tile([P, 1], BF, tag="zrb", name=f"zrb{it}")
                nc.vector.tensor_copy(zrb[:], zr[:])
                ex_hist[it % 2] = ex
                zrb_hist[it % 2] = zrb
            colsum_mms(NT - 1)

            # ---------------- tail -------------------------------------------
            # per jc-chunk: 1-row csum evac -> 4 transposes -> colsb columns
            # -> 4 scales (DVE-heavy; ScalarE copies are 2.5x slower) -> 4
            # single-tile stores, triggers alternating gpsimd/sync queues.
            colT = ptp.tile([P, NT], FP, tag="tp", name="colT")
            for jc in range(JC):
                src = cs_all[32 * jc : 32 * jc + 1, :]
                dst = csum_sb[32 * jc : 32 * jc + 1, :]
                if jc % 2 == 0:
                    nc.vector.tensor_copy(dst, src)
                else:
                    nc.scalar.copy(dst, src)
            for jc in range(JC):
                for bb in range(4):
                    jt = 4 * jc + bb
                    nc.tensor.matmul(
                        colT[:, jt : jt + 1],
                        lhsT=csum_sb[32 * jc : 32 * jc + 1, bb * P : (bb + 1) * P],
                        rhs=idf[32 * jc : 32 * jc + 1, :],
                        is_transpose=True,
                        skip_group_check=True,
                        tile_position=(32 * jc, 0),
                    )
                nc.vector.tensor_copy(
                    colsb[:, 4 * jc : 4 * jc + 4], colT[:, 4 * jc : 4 * jc + 4]
                )
                ob = obuf.tile([P, 4, D], BF, tag="ob", name=f"ob{jc}")
                for bb in range(4):
                    jt = 4 * jc + bb
                    csl = colsb[:, jt : jt + 1]
                    if bb == 1:
                        nc.scalar.mul(ob[:, bb, :], Y[:, jt, :], csl)
                    else:
                        nc.vector.tensor_scalar_mul(ob[:, bb, :], Y[:, jt, :], csl)
                q = nc.gpsimd if jc % 2 == 0 else nc.sync
                q.dma_start(ov[:, 4 * jc : 4 * jc + 4, :], ob[:])

    nc.compile()
    return nc


_NC_CACHE = None


def _get_nc():
    global _NC_CACHE
    if _NC_CACHE is None:
        _NC_CACHE = _build()
    return _NC_CACHE


BF_NP = ml_dtypes.bfloat16


def _prep(inputs):
    ln_w = np.asarray(inputs["ln_w"], dtype=np.float32)
    ln_b = np.asarray(inputs["ln_b"], dtype=np.float32)
    Wq = np.asarray(inputs["Wq"], dtype=np.float32)
    Wkv = np.asarray(inputs["Wkv"], dtype=np.float32)
    Wout = np.asarray(inputs["Wout"], dtype=np.float32)

    def permute_rows(w):  # row (kt*P + p) -> row (p*KT + kt) for big packets
        ct = w.shape[0] // P
        return np.ascontiguousarray(
            w.reshape(ct, P, w.shape[1]).transpose(1, 0, 2).reshape(w.shape)
        )

    wq_f = Wq * (SCALE * ln_w)[:, None]
    wq_h = permute_rows(wq_f.astype(BF_NP))
    wk_h = permute_rows(np.ascontiguousarray(Wkv[:, :E]).astype(BF_NP))
    w2_h = permute_rows((Wkv[:, E:] @ Wout).astype(BF_NP))
    wqq_h = np.ascontiguousarray(
        np.stack([wq_f.sum(0), SCALE * (ln_b @ Wq)]).astype(BF_NP)
    )

    def t_chunks(a):  # [2048, 1024] -> [(c p kt), i'] = [4096, 512]
        at = np.ascontiguousarray(a.astype(BF_NP).T)          # [D, n]
        return np.ascontiguousarray(
            at.reshape(KT, P, CH, F).transpose(2, 1, 0, 3).reshape(CH * D, F)
        )

    xs = np.asarray(inputs["x"], dtype=np.float32)
    ms = np.asarray(inputs["media"], dtype=np.float32)
    shared = {"wq": wq_h, "wk": wk_h, "w2": w2_h, "wqq": wqq_h}
    return [
        dict(shared, xt=t_chunks(xs[b]), mt=t_chunks(ms[b])) for b in range(B)
    ]


def _unscramble(o):  # [2048, 1024] HBM rows p*16+jt -> position rows jt*128+p
    return np.ascontiguousarray(
        o.reshape(P, NT, D).transpose(1, 0, 2).reshape(M, D)
    ).astype(np.float32)


def _run(inputs, trace=False, **kw):
    nc = _get_nc()
    in_maps = _prep(inputs)
    res = run_bass_kernel_spmd(nc, in_maps, core_ids=list(range(B)), trace=trace, **kw)
    out = np.stack(
        [_unscramble(res.results[b]["out"]) for b in range(B)], axis=0
    )
    return out, res


def kernel(**inputs) -> np.ndarray:
    out, _ = _run(inputs, trace=False)
    return out


# revision 18
# speedup vs baseline: 1.2596x; 1.0122x over previous
"""Trainium2 Bass kernel for nn_CrossAttention_47004122087816.

Math (faithful to the reference's "buggy einsum"):
    xn   = LayerNorm(x); xnb = xn * ln_w + ln_b
    q    = (xnb @ Wq) * SCALE            [n, E]
    k, v = split(media @ Wkv)            [m, E] each
    sim  = q @ k^T                       [n, m]
    colsum[j] = sum_i softmax(sim, -1)[i, j]
    out[j, :] = colsum[j] * (v @ Wout)[j, :]

Sharding: pure data-parallel - batch b=8 over 8 NeuronCores.

v2 redesign (vs the DMA-transpose baseline):
 - x and media are transposed on the HOST (layout-only prep, like the
   existing weight-row permutation), so the device loads land directly in
   the [D-part, rows-free] layout every matmul wants.  This removes all 32
   on-chip dma_start_transpose ops (~8MB of DMA traffic and the 35us
   startup serialization they caused).
 - LayerNorm is restructured to work in the transposed layout:
     q_i = r_i * (x_i @ wq' - mu_i * colsum(wq') + sigma_i * q0)
   with wq' = Wq * ln_w * SCALE, q0 = SCALE * ln_b @ Wq.  Sx and Sxx come
   from ones-vector matmuls (cheap M=1 PE work), the rank-1 corrections are
   K=1 matmuls accumulated into the q PSUM groups, and the final per-row
   scale r_i is folded into the sim-phase Exp activation's per-partition
   `scale` operand - zero extra elementwise passes over q.
 - v is never materialized: W2 = Wkv_v @ Wout is folded on the host, and
   Y = media @ W2 is computed directly (same FLOPs, one less PSUM
   evacuation pass and 2MB less SBUF).
 - sigma rows are flipped to per-partition columns with 16 tiny PE
   transposes (rhs = 1x1 identity); same trick turns the colsum PSUM rows
   into per-partition scalars in the tail, replacing 16 single-column
   scatter DMAs.
 - fp8 was evaluated (DoubleRow would halve PE time) and rejected: exp()
   amplification puts even k-only fp8 at ~2e-2 rel err, the whole gate.
 - Tail: colsum transposes -> one [128,16] copy -> 16 scaled copies
   (DVE/ScalarE alternating) -> paired 4KB-descriptor SWDGE stores.
"""

import sys

for _p in ("/opt/trn_rl_repo",):
    if _p not in sys.path:
        sys.path.insert(0, _p)

import numpy as np
import ml_dtypes

import concourse.bass as bass  # noqa: F401
import concourse.tile as tile
from concourse import bacc, mybir
from concourse.bass_utils import run_bass_kernel_spmd

B = 8
N = 2048          # x rows per batch element
M = 2048          # media rows per batch element
D = 1024          # model dim
E = 512           # inner dim
P = 128           # partitions
F = 512           # one PSUM bank of fp32
KT = D // P       # 8  contraction tiles over model dim
ET = E // P       # 4  contraction tiles over inner dim
NT = N // P       # 16 row tiles (positions)
JC = M // F       # 4  column chunks of 512
CH = N // F       # 4  position chunks of 512
SCALE = 64 ** -0.5
EPS = 1e-5

FP = mybir.dt.float32
BF = mybir.dt.bfloat16

AF = mybir.ActivationFunctionType
ALU = mybir.AluOpType
AX = mybir.AxisListType


def _build():
    nc = bacc.Bacc("TRN2", target_bir_lowering=False, debug=False, num_devices=B)

    # host layouts (see _run): xt/mt row (c*1024 + p*8 + kt) col i' holds
    # x[c*512 + i', kt*128 + p] -> per-partition 8KB contiguous loads.
    xt = nc.dram_tensor("xt", [CH * D, F], BF, kind="ExternalInput").ap()
    mt = nc.dram_tensor("mt", [CH * D, F], BF, kind="ExternalInput").ap()
    wq = nc.dram_tensor("wq", [D, E], BF, kind="ExternalInput").ap()
    wk = nc.dram_tensor("wk", [D, E], BF, kind="ExternalInput").ap()
    w2 = nc.dram_tensor("w2", [D, D], BF, kind="ExternalInput").ap()
    wqq = nc.dram_tensor("wqq", [2, E], BF, kind="ExternalInput").ap()
    out = nc.dram_tensor("out", [M, D], BF, kind="ExternalOutput").ap()

    xtv = xt.rearrange("(c p kt) i -> c p kt i", p=P, kt=KT)
    mtv = mt.rearrange("(c p kt) i -> c p kt i", p=P, kt=KT)
    # store tile jt partition p -> HBM row p*16+jt (host unscrambles);
    # paired stores give 4KB contiguous per-partition descriptors.
    ov = out.rearrange("(p t) d -> p t d", t=NT)

    with tile.TileContext(nc) as tc:
        from contextlib import ExitStack

        with ExitStack() as ctx:
            consts = ctx.enter_context(tc.tile_pool(name="consts", bufs=1))
            acts = ctx.enter_context(tc.tile_pool(name="acts", bufs=1))
            mtp = ctx.enter_context(tc.tile_pool(name="mtp", bufs=4))
            xtp = ctx.enter_context(tc.tile_pool(name="xtp", bufs=2))
            sqp = ctx.enter_context(tc.tile_pool(name="sqp", bufs=1))
            rows = ctx.enter_context(tc.tile_pool(name="rows", bufs=2))
            expp = ctx.enter_context(tc.tile_pool(name="expp", bufs=2))
            zsp = ctx.enter_context(tc.tile_pool(name="zsp", bufs=2))
            zrbp = ctx.enter_context(tc.tile_pool(name="zrbp", bufs=2))
            obuf = ctx.enter_context(tc.tile_pool(name="obuf", bufs=5))
            pmm = ctx.enter_context(tc.tile_pool(name="pmm", bufs=3, space="PSUM"))
            pyy = ctx.enter_context(tc.tile_pool(name="pyy", bufs=1, space="PSUM"))
            pst = ctx.enter_context(tc.tile_pool(name="pst", bufs=1, space="PSUM"))
            ptp = ctx.enter_context(tc.tile_pool(name="ptp", bufs=1, space="PSUM"))
            pcs = ctx.enter_context(tc.tile_pool(name="pcs", bufs=1, space="PSUM"))

            wq_t = consts.tile([P, KT, E], BF)
            wk_t = consts.tile([P, KT, E], BF)
            w2_t = consts.tile([P, KT, D], BF)
            wqq_t = consts.tile([2, E], BF)
            wdum = consts.tile([P, F], BF)
            ones_t = consts.tile([P, 1], BF)
            idf = consts.tile([P, 1], FP)     # 1x1 identity slices for transposes
            eps_t = consts.tile([1, 1], FP)
            r_sb = consts.tile([P, NT], FP)   # 1/sigma per position column
            colsb = consts.tile([P, NT], FP)
            csum_sb = consts.tile([P, F], FP)

            kT = acts.tile([P, ET, M], BF)
            qT = acts.tile([P, ET, N], BF)
            Y = acts.tile([P, NT, D], BF)

            nc.vector.memset(ones_t[:], 1.0)
            nc.vector.memset(idf[:], 1.0)
            nc.vector.memset(eps_t[:], EPS)
            nc.vector.memset(wdum[:], 0.125)
            # PE p-state warm-up during the initial DMA window
            for wdi in range(5):
                pd = pmm.tile([P, F], FP, tag="ps", name=f"warm{wdi}")
                nc.tensor.matmul(
                    pd[:], lhsT=wdum[:, 0:P], rhs=wdum[:], start=True, stop=True
                )

            # ---------------- bulk loads -------------------------------------
            mts: list = []
            xts: list = []

            def load_m(c):
                t = mtp.tile([P, KT, F], BF, tag="mt", name=f"mt{c}")
                nc.gpsimd.dma_start(t[:], mtv[c])
                mts.append(t)

            def load_x(c):
                t = xtp.tile([P, KT, F], BF, tag="xt", name=f"xt{c}")
                nc.gpsimd.dma_start(t[:], xtv[c])
                xts.append(t)

            # SWDGE in need-order; wk/wq/wqq on scalar HWDGE; w2 on sync,
            # emitted after k_chunk(0) so k0's queue-semaphore wait
            # does not cover it.
            wkv = wk.rearrange("(p kt) e -> p kt e", kt=KT)
            nc.scalar.dma_start(wk_t[:, 0 : KT // 2, :], wkv[:, 0 : KT // 2, :])
            nc.scalar.dma_start(wk_t[:, KT // 2 :, :], wkv[:, KT // 2 :, :])
            # mt0 in halves, first on SWDGE (so e0's kt0-3 matmuls can
            # start while the second half still streams).
            t0 = mtp.tile([P, KT, F], BF, tag="mt", name="mt0")
            nc.gpsimd.dma_start(t0[:, 0 : KT // 2, :], mtv[0][:, 0 : KT // 2, :])
            nc.gpsimd.dma_start(t0[:, KT // 2 :, :], mtv[0][:, KT // 2 :, :])
            mts.append(t0)
            load_x(0)
            load_m(1)
            load_x(1)
            load_m(2)
            load_x(2)
            load_m(3)
            load_x(3)
            # w2 last on SWDGE: streams only after all x/media loads, so it
            # never competes with feed-critical traffic; ready well before
            # the first y_group.
            nc.gpsimd.dma_start(
                w2_t[:], w2.rearrange("(p kt) d -> p kt d", kt=KT)
            )
            # ---------------- feed helpers -----------------------------------
            def k_chunk(c):
                for e in range(ET):
                    ps = pmm.tile([P, F], FP, tag="ps", name=f"k{c}_{e}")
                    for kt in range(KT):
                        nc.tensor.matmul(
                            ps[:],
                            lhsT=wk_t[:, kt, e * P : (e + 1) * P],
                            rhs=mts[c][:, kt, :],
                            start=(kt == 0),
                            stop=(kt == KT - 1),
                        )
                    dst = kT[:, e, c * F : (c + 1) * F]
                    if e % 2 == 0:
                        nc.scalar.copy(dst, ps[:])
                    else:
                        nc.vector.tensor_copy(dst, ps[:])

            def stats_chunk(c):
                sq = sqp.tile([P, KT, F], BF, tag="sq", name=f"sq{c}")
                nc.vector.tensor_tensor(sq[:], xts[c][:], xts[c][:], ALU.mult)
                Sx = pst.tile([1, F], FP, tag="sx", name=f"sx{c}")
                Sxx = pst.tile([1, F], FP, tag="sxx", name=f"sxx{c}")
                for kt in range(KT):
                    nc.tensor.matmul(
                        Sx[:],
                        lhsT=ones_t[:],
                        rhs=xts[c][:, kt, :],
                        start=(kt == 0),
                        stop=(kt == KT - 1),
                    )
                for kt in range(KT):
                    nc.tensor.matmul(
                        Sxx[:],
                        lhsT=ones_t[:],
                        rhs=sq[:, kt, :],
                        start=(kt == 0),
                        stop=(kt == KT - 1),
                    )
                # row math: -mu (bf16), mu^2, var, sigma (f32 + bf16)
                m2 = rows.tile([1, F], FP, tag="m2", name=f"m2{c}")
                nc.scalar.activation(
                    m2[:], Sx[:], func=AF.Square, bias=0.0, scale=1.0 / D
                )
                vt1 = rows.tile([1, F], FP, tag="vt1", name=f"vt1{c}")
                nc.vector.tensor_scalar(vt1[:], Sxx[:], 1.0 / D, None, ALU.mult)
                varx = rows.tile([1, F], FP, tag="varx", name=f"varx{c}")
                nc.vector.tensor_tensor(varx[:], vt1[:], m2[:], ALU.subtract)
                sgf = rows.tile([1, F], FP, tag="sgf", name=f"sgf{c}")
                nc.scalar.activation(
                    sgf[:], varx[:], func=AF.Sqrt, bias=eps_t[:], scale=1.0
                )
                # stack [-mu; sigma] on partitions 0/1 for one K=2 rank-1
                st2 = rows.tile([2, F], BF, tag="st2", name=f"st2{c}")
                nc.scalar.activation(
                    st2[0:1, :], Sx[:], func=AF.Copy, bias=0.0, scale=-1.0 / D
                )
                sgb = rows.tile([1, F], BF, tag="sgb", name=f"sgb{c}")
                nc.scalar.activation(
                    sgb[:], varx[:], func=AF.Sqrt, bias=eps_t[:], scale=1.0
                )
                nc.sync.dma_start(st2[1:2, :], sgb[:])
                return st2, sgf

            def sig_transpose(c, sgf):
                # [1,512] sigma row -> r_sb[:, 4c:4c+4] columns via 4 tiny
                # PE transposes (rhs = 1x1 identity) + one PSUM reciprocal.
                pt = ptp.tile([P, CH], FP, tag="tp", name=f"sigT{c}")
                for u in range(CH):
                    nc.tensor.matmul(
                        pt[:, u : u + 1],
                        lhsT=sgf[0:1, u * P : (u + 1) * P],
                        rhs=idf[0:1, :],
                        is_transpose=True,
                        skip_group_check=True,
                        tile_position=(0, 0),
                    )
                nc.vector.reciprocal(r_sb[:, 4 * c : 4 * c + 4], pt[:])

            def q_chunk(c, st2):
                for e in range(ET):
                    ps = pmm.tile([P, F], FP, tag="ps", name=f"q{c}_{e}")
                    for kt in range(KT):
                        nc.tensor.matmul(
                            ps[:],
                            lhsT=wq_t[:, kt, e * P : (e + 1) * P],
                            rhs=xts[c][:, kt, :],
                            start=(kt == 0),
                            stop=False,
                        )
                    # K=2 rank-1: - mu (x) wqsum  +  sigma (x) q0
                    nc.tensor.matmul(
                        ps[:],
                        lhsT=wqq_t[0:2, e * P : (e + 1) * P],
                        rhs=st2[0:2, :],
                        start=False,
                        stop=True,
                    )
                    dst = qT[:, e, c * F : (c + 1) * F]
                    if e % 2 == 0:
                        nc.vector.tensor_copy(dst, ps[:])
                    else:
                        nc.scalar.copy(dst, ps[:])

            # ---- feed: PE order k0 s0 k1 T0 q0 s1 k2 T1 q1 s2 k3 T2 q2 s3 T3 q3
            k_chunk(0)
            nc.scalar.dma_start(wq_t[:], wq.rearrange("(p kt) e -> p kt e", kt=KT))
            nc.scalar.dma_start(wqq_t[:], wqq)
            sta = stats_chunk(0)
            k_chunk(1)
            sig_transpose(0, sta[1])
            q_chunk(0, sta[0])
            stb = stats_chunk(1)
            k_chunk(2)
            sig_transpose(1, stb[1])
            q_chunk(1, stb[0])
            stc = stats_chunk(2)
            k_chunk(3)
            sig_transpose(2, stc[1])
            q_chunk(2, stc[0])
            std = stats_chunk(3)
            sig_transpose(3, std[1])
            q_chunk(3, std[0])

            # ---------------- sim, exp (+z via accum), colsum, Y -------------
            cs_all = pcs.tile([P, F], FP)
            nc.vector.memset(cs_all[:], 0.0)
            ex_hist: list = [None, None]
            zrb_hist: list = [None, None]

            def colsum_mms(it):
                ex_t = ex_hist[it % 2]
                zrb_t = zrb_hist[it % 2]
                for jc in range(JC):
                    nc.tensor.matmul(
                        cs_all[32 * jc : 32 * jc + 1, :],
                        lhsT=zrb_t[:],
                        rhs=ex_t[:, jc * F : (jc + 1) * F],
                        start=(it == 0),
                        stop=(it == NT - 1),
                        skip_group_check=True,
                        tile_position=(0, 32 * jc),
                    )

            def sim_group(it, jc, ex, zpart):
                ps = pmm.tile([P, F], FP, tag="ps", name=f"sim{it}_{jc}")
                for et in range(ET):
                    nc.tensor.matmul(
                        ps[:],
                        lhsT=qT[:, et, it * P : (it + 1) * P],
                        rhs=kT[:, et, jc * F : (jc + 1) * F],
                        start=(et == 0),
                        stop=(et == ET - 1),
                    )
                nc.scalar.activation(
                    ex[:, jc * F : (jc + 1) * F],
                    ps[:],
                    func=AF.Exp,
                    bias=0.0,
                    scale=r_sb[:, it : it + 1],
                    accum_out=zpart[:, jc : jc + 1],
                )

            def y_group(g):
                c, mb, dh = g // 8, (g % 8) // 2, g % 2
                jt = 4 * c + mb
                psn = pyy.tile([P, F], FP, tag="py", name=f"y{g}")
                for kt in range(KT):
                    nc.tensor.matmul(
                        psn[:],
                        lhsT=mts[c][:, kt, mb * P : (mb + 1) * P],
                        rhs=w2_t[:, kt, dh * F : (dh + 1) * F],
                        start=(kt == 0),
                        stop=(kt == KT - 1),
                    )
                nc.vector.tensor_copy(Y[:, jt, dh * F : (dh + 1) * F], psn[:])

            for it in range(NT):
                ex = expp.tile([P, M], BF, tag="ex", name=f"ex{it}")
                zpart = zsp.tile([P, JC], FP, tag="zpt", name=f"zpt{it}")
                sim_group(it, 0, ex, zpart)
                sim_group(it, 1, ex, zpart)
                y_group(2 * it)
                sim_group(it, 2, ex, zpart)
                if it > 0:
                    colsum_mms(it - 1)
                sim_group(it, 3, ex, zpart)
                y_group(2 * it + 1)
                z = zsp.tile([P, 1], FP, tag="z", name=f"z{it}")
                nc.vector.tensor_reduce(z[:], zpart[:], axis=AX.X, op=ALU.add)
                zr = zsp.tile([P, 1], FP, tag="zr", name=f"zr{it}")
                nc.vector.reciprocal(zr[:], z[:])
                zrb = zrbp.tile([P, 1], BF, tag="zrb", name=f"zrb{it}")
                nc.vector.tensor_copy(zrb[:], zr[:])
                ex_hist[it % 2] = ex
                zrb_hist[it % 2] = zrb
            colsum_mms(NT - 1)

            # ---------------- tail -------------------------------------------
            # per jc-chunk: 1-row csum evac -> 4 transposes -> colsb columns
            # -> 4 scales (DVE-heavy; ScalarE copies are 2.5x slower) -> 4
            # single-tile stores, triggers alternating gpsimd/sync queues.
            colT = ptp.tile([P, NT], FP, tag="tp", name="colT")
            for jc in range(JC):
                src = cs_all[32 * jc : 32 * jc + 1, :]
                dst = csum_sb[32 * jc : 32 * jc + 1, :]
                if jc % 2 == 0:
                    nc.vector.tensor_copy(dst, src)
                else:
                    nc.scalar.copy(dst, src)
            for jc in range(JC):
                for bb in range(4):
                    jt = 4 * jc + bb
                    nc.tensor.matmul(
                        colT[:, jt : jt + 1],
                        lhsT=csum_sb[32 * jc : 32 * jc + 1, bb * P : (bb + 1) * P],
                        rhs=idf[32 * jc : 32 * jc + 1, :],
                        is_transpose=True,
                        skip_group_check=True,
                        tile_position=(32 * jc, 0),
                    )
                nc.vector.tensor_copy(
                    colsb[:, 4 * jc : 4 * jc + 4], colT[:, 4 * jc : 4 * jc + 4]
                )
                for bb in range(4):
                    jt = 4 * jc + bb
                    ob = obuf.tile([P, D], BF, tag="ob", name=f"ob{jt}")
                    csl = colsb[:, jt : jt + 1]
                    if bb == 1:
                        nc.scalar.mul(ob[:], Y[:, jt, :], csl)
                    else:
                        nc.vector.tensor_scalar_mul(ob[:], Y[:, jt, :], csl)
                    q = nc.gpsimd if jt % 2 == 0 else nc.sync
                    q.dma_start(ov[:, jt, :], ob[:])

    nc.compile()
    return nc


_NC_CACHE = None


def _get_nc():
    global _NC_CACHE
    if _NC_CACHE is None:
        _NC_CACHE = _build()
    return _NC_CACHE


BF_NP = ml_dtypes.bfloat16


def _prep(inputs):
    ln_w = np.asarray(inputs["ln_w"], dtype=np.float32)
    ln_b = np.asarray(inputs["ln_b"], dtype=np.float32)
    Wq = np.asarray(inputs["Wq"], dtype=np.float32)
    Wkv = np.asarray(inputs["Wkv"], dtype=np.float32)
    Wout = np.asarray(inputs["Wout"], dtype=np.float32)

    def permute_rows(w):  # row (kt*P + p) -> row (p*KT + kt) for big packets
        ct = w.shape[0] // P
        return np.ascontiguousarray(
            w.reshape(ct, P, w.shape[1]).transpose(1, 0, 2).reshape(w.shape)
        )

    wq_f = Wq * (SCALE * ln_w)[:, None]
    wq_h = permute_rows(wq_f.astype(BF_NP))
    wk_h = permute_rows(np.ascontiguousarray(Wkv[:, :E]).astype(BF_NP))
    w2_h = permute_rows((Wkv[:, E:] @ Wout).astype(BF_NP))
    wqq_h = np.ascontiguousarray(
        np.stack([wq_f.sum(0), SCALE * (ln_b @ Wq)]).astype(BF_NP)
    )

    def t_chunks(a):  # [2048, 1024] -> [(c p kt), i'] = [4096, 512]
        at = np.ascontiguousarray(a.astype(BF_NP).T)          # [D, n]
        return np.ascontiguousarray(
            at.reshape(KT, P, CH, F).transpose(2, 1, 0, 3).reshape(CH * D, F)
        )

    xs = np.asarray(inputs["x"], dtype=np.float32)
    ms = np.asarray(inputs["media"], dtype=np.float32)
    shared = {"wq": wq_h, "wk": wk_h, "w2": w2_h, "wqq": wqq_h}
    return [
        dict(shared, xt=t_chunks(xs[b]), mt=t_chunks(ms[b])) for b in range(B)
    ]


def _unscramble(o):  # [2048, 1024] HBM rows p*16+jt -> position rows jt*128+p
    return np.ascontiguousarray(
        o.reshape(P, NT, D).transpose(1, 0, 2).reshape(M, D)
    ).astype(np.float32)


def _run(inputs, trace=False, **kw):
    nc = _get_nc()
    in_maps = _prep(inputs)
    res = run_bass_kernel_spmd(nc, in_maps, core_ids=list(range(B)), trace=trace, **kw)
    out = np.stack(
        [_unscramble(res.results[b]["out"]) for b in range(B)], axis=0
    )
    return out, res


def kernel(**inputs) -> np.ndarray:
    out, _ = _run(inputs, trace=False)
    return out
